# revision 1
# baseline (speedup 1.0000x reference)
"""Causal self-attention (B=4, T=2048, C=1024, H=16) on 8 trn2 NeuronCores.

Sharding: core = (batch b, head-group g) with b = core//2, g = core%2.
Each core handles one batch and 8 heads (column-parallel qkv, row-parallel
out_proj).  Cores return partial out-projection results; the host sums the
two head-group partials per batch and adds b_out (the "all-reduce" of the
row-parallel out_proj done host-side during unshard).

Per-core kernel (all matmuls bf16 with fp32 PSUM accumulate):
  - qk projection into d-major layout qkT [1024, T] (= W^T @ x^T), bias added
    per-partition at PSUM eviction.
  - v  projection into T-major layout V [T, 512], bias folded in via a K=1
    matmul with a ones stationary vector.
  - scores computed transposed: S^T[k, q] = kT_tile^T @ qT (contraction d=64;
    even/odd heads of a 128-partition block sit at base partitions 0/64 so
    the PE can row-tile them concurrently).
  - p = exp(0.125 * S^T) on the scalar engine (no max subtraction; logits are
    bounded for this data), causal masking by multiplying diagonal-straddling
    tiles with precomputed 0/1 masks on the vector engine.
  - y^T (+ softmax denominators) via one fused matmul with lhsT = [V | 1].
  - normalization: reciprocal on DVE, partition-broadcast via a K=1 ones
    matmul on the PE, multiply on DVE, small SBUF->SBUF DMA to place odd
    heads at partitions 64..127 of the y^T store.
  - out projection: out[t, c] accumulated over 4 hd-blocks of y^T @ w_out.
"""

import os
import numpy as np

try:
    import concourse.bass as bass
except ImportError:  # pragma: no cover
    import sys

    sys.path.insert(0, "/opt/trn_rl_repo")
    import concourse.bass as bass

import concourse.mybir as mybir
from concourse.bacc import Bacc
from concourse.bass_utils import run_bass_kernel_spmd
from concourse.tile import TileContext
from ml_dtypes import bfloat16

B, T, C, H = 4, 2048, 1024, 16
HD = 64        # head dim
G = 512        # head-group width: 8 heads * 64
P = 128
NCT = C // P   # contraction tiles over C
NTT = T // P   # 128-row tiles over T
CHW = 512      # q-chunk width
NCH = T // CHW
SPAN = 3       # k-tiles per S^T span (one PSUM tile = 3 banks)

F32 = mybir.dt.float32
BF16 = mybir.dt.bfloat16

_BUILT = None
LAST_EXEC_TIME_NS = None


def _build_bass(reps=1):
    nc = Bacc("TRN2", target_bir_lowering=False)

    xt_d = nc.dram_tensor("xt", [C, T], BF16, kind="ExternalInput")
    wqk_d = nc.dram_tensor("wqk", [C, 2 * G], BF16, kind="ExternalInput")
    bqk_d = nc.dram_tensor("bqk", [P, 8], F32, kind="ExternalInput")
    wv_d = nc.dram_tensor("wv", [C, G], BF16, kind="ExternalInput")
    bv_d = nc.dram_tensor("bv", [1, G], BF16, kind="ExternalInput")
    wo_d = nc.dram_tensor("wo", [P, 4, C], BF16, kind="ExternalInput")
    mask_d = nc.dram_tensor("mask", [P, 4, CHW], BF16, kind="ExternalInput")
    out_d = nc.dram_tensor("out", [T, C], F32, kind="ExternalOutput")

    Exp = mybir.ActivationFunctionType.Exp

    with TileContext(nc) as tc:
        with (
            tc.tile_pool(name="const", bufs=1) as cpool,
            tc.tile_pool(name="work", bufs=3) as wpool,
            tc.tile_pool(name="ppool", bufs=4) as ppool,
            tc.tile_pool(name="pspool", bufs=2, space="PSUM") as pspool,
            tc.tile_pool(name="avpool", bufs=2, space="PSUM") as avpool,
        ):
            xts = cpool.tile([P, NCT, T], BF16, tag="xts")
            wqks = cpool.tile([P, NCT, 2 * G], BF16, tag="wqks")
            bqks = cpool.tile([P, 8], F32, tag="bqks")
            wvs = cpool.tile([P, NCT, G], BF16, tag="wvs")
            bvs = cpool.tile([1, G], BF16, tag="bvs")
            wos = cpool.tile([P, 4, C], BF16, tag="wos")
            masks = cpool.tile([P, 4, CHW], BF16, tag="masks")
            ones = cpool.tile([1, P], BF16, tag="ones")
            ones64 = cpool.tile([P, HD], F32, tag="ones64")  # row 64 = 1.0
            qkts = cpool.tile([P, 8, T], BF16, tag="qkts")
            vs = cpool.tile([P, NTT, 8, HD + 1], BF16, tag="vs")
            yts = cpool.tile([P, 4, T], BF16, tag="yts")

            nc.gpsimd.dma_start(out=xts[:, :, :], in_=xt_d.rearrange("(a p) t -> p a t", p=P))
            nc.gpsimd.dma_start(out=wqks[:, :, :], in_=wqk_d.rearrange("(a p) m -> p a m", p=P))
            nc.gpsimd.dma_start(out=bqks[:, :], in_=bqk_d[:, :])
            nc.gpsimd.dma_start(out=wvs[:, :, :], in_=wv_d.rearrange("(a p) m -> p a m", p=P))
            nc.gpsimd.dma_start(out=bvs[:, :], in_=bv_d[:, :])
            nc.gpsimd.dma_start(out=wos[:, :, :], in_=wo_d[:, :, :])
            nc.gpsimd.dma_start(out=masks[:, :, :], in_=mask_d[:, :, :])
            nc.vector.memset(ones[:, :], 1.0)
            nc.vector.memset(ones64[HD:HD + 1, :], 1.0)

            for rep in range(reps):
                # ---- V phase: V[t, (h d)] = x @ wv + bv, T-major ----
                for tt in range(NTT):
                    ps = pspool.tile([P, 3 * CHW], F32, tag="s")
                    pv = ps[:, 0:G]
                    for ct in range(NCT):
                        nc.tensor.matmul(
                            pv,
                            lhsT=xts[:, ct, tt * P:(tt + 1) * P],
                            rhs=wvs[:, ct, :],
                            start=(ct == 0),
                            stop=False,
                        )
                    nc.tensor.matmul(pv, lhsT=ones[:, :], rhs=bvs[:, :], start=False, stop=True)
                    nc.vector.memset(vs[:, tt, :, HD:HD + 1], 1.0)
                    nc.vector.tensor_copy(
                        out=vs[:, tt, :, 0:HD],
                        in_=pv.rearrange("p (h d) -> p h d", h=8),
                    )

                # ---- per head-pair: qk projection, then attention ----
                for hp in range(4):
                    for mt in (hp, 4 + hp):  # q block then k block
                        for ch in range(NCH):
                            ps = pspool.tile([P, 3 * CHW], F32, tag="s")
                            pq = ps[:, 0:CHW]
                            for ct in range(NCT):
                                nc.tensor.matmul(
                                    pq,
                                    lhsT=wqks[:, ct, mt * P:(mt + 1) * P],
                                    rhs=xts[:, ct, ch * CHW:(ch + 1) * CHW],
                                    start=(ct == 0),
                                    stop=(ct == NCT - 1),
                                )
                            nc.vector.tensor_scalar_add(
                                out=qkts[:, mt, ch * CHW:(ch + 1) * CHW],
                                in0=pq,
                                scalar1=bqks[:, mt:mt + 1],
                            )

                    for ch in range(NCH):
                        nkt = 4 * ch + 4
                        avs = {}
                        for par in (0, 1):
                            avs[par] = avpool.tile([P, CHW], F32, tag="av", name=f"av_{rep}_{hp}_{ch}_{par}")
                        for s0 in range(0, nkt, SPAN):
                            ns = min(SPAN, nkt - s0)
                            sps = {}
                            pts = {}
                            for par in (0, 1):
                                po = 64 * par
                                sp = pspool.tile([P, 3 * CHW], F32, tag="s", name=f"sp_{rep}_{hp}_{ch}_{s0}_{par}")
                                for i in range(ns):
                                    kt = s0 + i
                                    nc.tensor.matmul(
                                        sp[:, i * CHW:(i + 1) * CHW],
                                        lhsT=qkts[po:po + 64, 4 + hp, kt * P:(kt + 1) * P],
                                        rhs=qkts[po:po + 64, hp, ch * CHW:(ch + 1) * CHW],
                                        start=True,
                                        stop=True,
                                    )
                                sps[par] = sp
                            for par in (0, 1):
                                pt = ppool.tile([P, 3 * CHW], BF16, tag="p", name=f"pt_{rep}_{hp}_{ch}_{s0}_{par}")
                                nc.scalar.activation(
                                    out=pt[:, 0:ns * CHW],
                                    in_=sps[par][:, 0:ns * CHW],
                                    func=Exp,
                                    scale=0.125,
                                )
                                for i in range(ns):
                                    kt = s0 + i
                                    if kt >= 4 * ch:
                                        r = kt - 4 * ch
                                        nc.vector.tensor_mul(
                                            out=pt[:, i * CHW:(i + 1) * CHW],
                                            in0=pt[:, i * CHW:(i + 1) * CHW],
                                            in1=masks[:, r, :],
                                        )
                                pts[par] = pt
                            for par in (0, 1):
                                hl = 2 * hp + par
                                for i in range(ns):
                                    kt = s0 + i
                                    nc.tensor.matmul(
                                        avs[par][0:HD + 1, :],
                                        lhsT=vs[:, kt, hl, :],
                                        rhs=pts[par][:, i * CHW:(i + 1) * CHW],
                                        start=(kt == 0),
                                        stop=(kt == nkt - 1),
                                    )
                        # normalize y^T by the softmax denominator (row HD of av)
                        for par in (0, 1):
                            yun = wpool.tile([P, CHW], F32, tag="yun")
                            nc.vector.tensor_copy(out=yun[0:HD + 1, :], in_=avs[par][0:HD + 1, :])
                            nc.vector.reciprocal(out=yun[HD:HD + 1, :], in_=yun[HD:HD + 1, :])
                            # broadcast the reciprocal row across 64 partitions with
                            # a K=1 matmul (ones column at base partition 64 to
                            # match the rhs base).
                            recb = avpool.tile([P, CHW], F32, tag="av", name=f"recb_{rep}_{hp}_{ch}_{par}")
                            nc.tensor.matmul(
                                recb[0:HD, :],
                                lhsT=ones64[HD:HD + 1, :],
                                rhs=yun[HD:HD + 1, :],
                                start=True,
                                stop=True,
                            )
                            ynorm = wpool.tile([HD, CHW], BF16, tag="ynorm")
                            nc.vector.tensor_mul(out=ynorm[:, :], in0=yun[0:HD, :], in1=recb[0:HD, :])
                            nc.gpsimd.dma_start(
                                out=yts[64 * par:64 * par + 64, hp, ch * CHW:(ch + 1) * CHW],
                                in_=ynorm[:, :],
                            )

                # ---- out projection: out[t, c] = sum_j yts[:, j] ^T @ wo[:, j] ----
                for tt in range(NTT):
                    for cc in range(2):
                        ps = pspool.tile([P, 3 * CHW], F32, tag="s")
                        po = ps[:, 0:CHW]
                        for j in range(4):
                            nc.tensor.matmul(
                                po,
                                lhsT=yts[:, j, tt * P:(tt + 1) * P],
                                rhs=wos[:, j, cc * CHW:(cc + 1) * CHW],
                                start=(j == 0),
                                stop=(j == 3),
                            )
                        ev = wpool.tile([P, CHW], F32, tag="ev")
                        nc.vector.tensor_copy(out=ev[:, :], in_=po)
                        nc.sync.dma_start(
                            out=out_d[tt * P:(tt + 1) * P, cc * CHW:(cc + 1) * CHW],
                            in_=ev[:, :],
                        )

    nc.finalize()
    return nc


def _make_masks():
    p = np.arange(P)[:, None]
    f = np.arange(CHW)[None, :]
    m = np.empty((P, 4, CHW), dtype=bfloat16)
    for r in range(4):
        m[:, r, :] = ((P * r + p) <= f).astype(bfloat16)
    return m


def _core_inputs(x, w_qkv, b_qkv, w_out, core, masks):
    b, g = core // 2, core % 2
    qc = slice(G * g, G * g + G)
    kc = slice(C + G * g, C + G * g + G)
    vc = slice(2 * C + G * g, 2 * C + G * g + G)
    xt = np.ascontiguousarray(x[b].T).astype(bfloat16)
    wqk = np.ascontiguousarray(
        np.concatenate([w_qkv[:, qc], w_qkv[:, kc]], axis=1)
    ).astype(bfloat16)
    bqk = np.ascontiguousarray(
        np.concatenate([b_qkv[qc], b_qkv[kc]]).reshape(8, P).T
    ).astype(np.float32)
    wv = np.ascontiguousarray(w_qkv[:, vc]).astype(bfloat16)
    bv = np.ascontiguousarray(b_qkv[vc].reshape(1, G)).astype(bfloat16)
    wo = np.ascontiguousarray(
        w_out[G * g:G * g + G, :].reshape(4, P, C).transpose(1, 0, 2)
    ).astype(bfloat16)
    return {
        "xt": xt,
        "wqk": wqk,
        "bqk": bqk,
        "wv": wv,
        "bv": bv,
        "wo": wo,
        "mask": masks,
    }


def kernel(x, w_qkv, b_qkv, w_out, b_out):
    global _BUILT, LAST_EXEC_TIME_NS
    x = np.asarray(x, dtype=np.float32)
    w_qkv = np.asarray(w_qkv, dtype=np.float32)
    b_qkv = np.asarray(b_qkv, dtype=np.float32)
    w_out = np.asarray(w_out, dtype=np.float32)
    b_out = np.asarray(b_out, dtype=np.float32)

    if _BUILT is None:
        _BUILT = _build_bass()
    nc = _BUILT

    masks = _make_masks()
    in_maps = [
        _core_inputs(x, w_qkv, b_qkv, w_out, core, masks) for core in range(8)
    ]
    trace = bool(int(os.environ.get("KERNEL_TRACE", "0")))
    res = run_bass_kernel_spmd(nc, in_maps, list(range(8)), trace=trace)
    LAST_EXEC_TIME_NS = res.exec_time_ns

    out = np.empty((B, T, C), dtype=np.float32)
    for b in range(B):
        out[b] = res.results[2 * b]["out"] + res.results[2 * b + 1]["out"] + b_out
    return out



# revision 3
# speedup vs baseline: 1.4101x; 1.4101x over previous
"""Causal self-attention (B=4, T=2048, C=1024, H=16) on 8 trn2 NeuronCores.

Sharding: core = (batch b, head-group g) with b = core//2, g = core%2.
Each core handles one batch and 8 heads (column-parallel qkv, row-parallel
out_proj).  Cores return partial out-projection results; the host sums the
two head-group partials per batch and adds b_out.

Per-core kernel (all matmuls bf16 with fp32 PSUM accumulate), restructured
for PE occupancy and minimal PE work:
  - streaming startup: x^T loaded in four T-chunks, w_qkv in eight 128-col
    blocks (HWDGE/SP queue), so the first projection matmuls start ~4us in.
  - per q-chunk ch (512 cols): qk projection for that chunk, V projection
    for its four 128-row tiles, then attention for the four head pairs, then
    the chunk's out-projection — out-proj and next-chunk projections are
    emitted as PE "filler" quanta interleaved between score spans so the PE
    stays busy while the Activation engine drains exp() work.
  - scores computed transposed S^T[k, q] with ragged causal widths (the
    k-tiles near the diagonal only compute the surviving q columns).
  - p = exp(0.125 * S^T) on the scalar engine on packed 2-k-tile spans;
    diagonal 128x128 triangles masked by a single precomputed 0/1 mask.
  - AV computed q-major: out[q, d] = sum_k P^T[k,q]^T V[k,d] with a fused
    ones column in V giving the softmax denominator in column 64 — output
    free size is 65 instead of 512, a ~2.3x cut in AV PE time.
  - normalization per-partition (reciprocal + tensor_scalar_mul on DVE),
    then a PE transpose (identity matmul) back to d-major y^T for the
    row-parallel out projection.
"""

import os
from collections import deque

import numpy as np

try:
    import concourse.bass as bass
except ImportError:  # pragma: no cover
    import sys

    sys.path.insert(0, "/opt/trn_rl_repo")
    import concourse.bass as bass

import concourse.mybir as mybir
from concourse.bacc import Bacc
from concourse.bass_utils import run_bass_kernel_spmd
from concourse.tile import TileContext
from ml_dtypes import bfloat16

B, T, C, H = 4, 2048, 1024, 16
HD = 64        # head dim
G = 512        # head-group width: 8 heads * 64
P = 128
NCT = C // P   # contraction tiles over C
NTT = T // P   # 128-row tiles over T
CHW = 512      # q-chunk width
NCH = T // CHW

F32 = mybir.dt.float32
BF16 = mybir.dt.bfloat16

_BUILT = None
LAST_EXEC_TIME_NS = None


def _widths(ch):
    """Per-k-tile ragged score widths and pt-column offsets for chunk ch."""
    nkt = 4 * ch + 4
    w = [CHW - max(0, P * (kt - 4 * ch)) for kt in range(nkt)]
    off = [0]
    for x in w:
        off.append(off[-1] + x)
    return nkt, w, off


PTW = _widths(NCH - 1)[2][-1]  # widest pt row (chunk 3): 7424


def _build_bass(reps=1):
    nc = Bacc("TRN2", target_bir_lowering=False)

    xt_d = nc.dram_tensor("xt", [C, T], BF16, kind="ExternalInput")
    wqk_d = nc.dram_tensor("wqk", [C, 2 * G], BF16, kind="ExternalInput")
    bqk_d = nc.dram_tensor("bqk", [P, 8], F32, kind="ExternalInput")
    wv_d = nc.dram_tensor("wv", [C, G], BF16, kind="ExternalInput")
    bv_d = nc.dram_tensor("bv", [1, G], BF16, kind="ExternalInput")
    wo_d = nc.dram_tensor("wo", [P, 4, C], BF16, kind="ExternalInput")
    tri_d = nc.dram_tensor("tri", [P, P], BF16, kind="ExternalInput")
    iden_d = nc.dram_tensor("iden", [P, P], BF16, kind="ExternalInput")
    out_d = nc.dram_tensor("out", [T, C], F32, kind="ExternalOutput")

    Exp = mybir.ActivationFunctionType.Exp

    with TileContext(nc) as tc:
        with (
            tc.tile_pool(name="const", bufs=1) as cpool,
            tc.tile_pool(name="work", bufs=3) as wpool,
            tc.tile_pool(name="bpool", bufs=2, space="PSUM") as bpool,
            tc.tile_pool(name="spool", bufs=2, space="PSUM") as spool,
            tc.tile_pool(name="apool", bufs=2, space="PSUM") as apool,
        ):
            xts = cpool.tile([P, NCT, T], BF16, tag="xts")
            wqks = cpool.tile([P, NCT, 2 * G], BF16, tag="wqks")
            bqks = cpool.tile([P, 8], F32, tag="bqks")
            wvs = cpool.tile([P, NCT, G], BF16, tag="wvs")
            bvs = cpool.tile([1, G], BF16, tag="bvs")
            wos = cpool.tile([P, 4, C], BF16, tag="wos")
            tri = cpool.tile([P, P], BF16, tag="tri")
            iden = cpool.tile([P, P], BF16, tag="iden")
            ones = cpool.tile([1, P], BF16, tag="ones")
            qkts = cpool.tile([P, 8, T], BF16, tag="qkts")
            vs = cpool.tile([P, NTT, 8, HD + 1], BF16, tag="vs")
            yts = cpool.tile([P, 4, T], BF16, tag="yts")
            pts = [
                cpool.tile([P, 2, PTW], BF16, tag=f"pt{i}", name=f"pt{i}")
                for i in range(2)
            ]

            xr = xt_d.rearrange("(a p) t -> p a t", p=P)
            wr = wqk_d.rearrange("(a p) m -> p a m", p=P)
            # DMA issue order ~ readiness order of first consumers.
            nc.sync.dma_start(out=xts[:, :, 0:CHW], in_=xr[:, :, 0:CHW])
            nc.sync.dma_start(out=wqks[:, :, 0:P], in_=wr[:, :, 0:P])
            nc.sync.dma_start(out=wqks[:, :, 4 * P:5 * P], in_=wr[:, :, 4 * P:5 * P])
            nc.sync.dma_start(out=bqks[:, :], in_=bqk_d[:, :])
            nc.sync.dma_start(out=tri[:, :], in_=tri_d[:, :])
            nc.sync.dma_start(out=iden[:, :], in_=iden_d[:, :])
            for mt in (1, 5, 2, 6, 3, 7):
                nc.sync.dma_start(
                    out=wqks[:, :, mt * P:(mt + 1) * P], in_=wr[:, :, mt * P:(mt + 1) * P]
                )
            nc.sync.dma_start(out=wvs[:, :, :], in_=wv_d.rearrange("(a p) m -> p a m", p=P))
            nc.sync.dma_start(out=bvs[:, :], in_=bv_d[:, :])
            for c in (1, 2, 3):
                nc.sync.dma_start(
                    out=xts[:, :, c * CHW:(c + 1) * CHW], in_=xr[:, :, c * CHW:(c + 1) * CHW]
                )
            nc.sync.dma_start(out=wos[:, :, :], in_=wo_d[:, :, :])
            nc.vector.memset(ones[:, :], 1.0)

            for rep in range(reps):
                rtag = f"r{rep}"

                def emit_qk(mt, ch):
                    ps = bpool.tile([P, CHW], F32, tag="B", name=f"qk_{rtag}_{mt}_{ch}")
                    for ct in range(NCT):
                        nc.tensor.matmul(
                            ps,
                            lhsT=wqks[:, ct, mt * P:(mt + 1) * P],
                            rhs=xts[:, ct, ch * CHW:(ch + 1) * CHW],
                            start=(ct == 0),
                            stop=(ct == NCT - 1),
                        )
                    nc.vector.tensor_scalar_add(
                        out=qkts[:, mt, ch * CHW:(ch + 1) * CHW],
                        in0=ps,
                        scalar1=bqks[:, mt:mt + 1],
                    )

                def emit_v(tt):
                    ps = bpool.tile([P, CHW], F32, tag="B", name=f"v_{rtag}_{tt}")
                    for ct in range(NCT):
                        nc.tensor.matmul(
                            ps,
                            lhsT=xts[:, ct, tt * P:(tt + 1) * P],
                            rhs=wvs[:, ct, :],
                            start=(ct == 0),
                            stop=False,
                        )
                    nc.tensor.matmul(ps, lhsT=ones[:, :], rhs=bvs[:, :], start=False, stop=True)
                    nc.vector.memset(vs[:, tt, :, HD:HD + 1], 1.0)
                    nc.vector.tensor_copy(
                        out=vs[:, tt, :, 0:HD],
                        in_=ps.rearrange("p (h d) -> p h d", h=8),
                    )

                def emit_out(tt, cc):
                    ps = bpool.tile([P, CHW], F32, tag="B", name=f"o_{rtag}_{tt}_{cc}")
                    for j in range(4):
                        nc.tensor.matmul(
                            ps,
                            lhsT=yts[:, j, tt * P:(tt + 1) * P],
                            rhs=wos[:, j, cc * CHW:(cc + 1) * CHW],
                            start=(j == 0),
                            stop=(j == 3),
                        )
                    ev = wpool.tile([P, CHW], F32, tag="ev", name=f"ev_{rtag}_{tt}_{cc}")
                    nc.vector.tensor_copy(out=ev[:, :], in_=ps)
                    nc.sync.dma_start(
                        out=out_d[tt * P:(tt + 1) * P, cc * CHW:(cc + 1) * CHW],
                        in_=ev[:, :],
                    )

                projq = deque()
                outq = deque()
                for mt in (0, 4, 1, 5, 2, 6, 3, 7):
                    projq.append((emit_qk, (mt, 0)))
                for tt in range(4):
                    projq.append((emit_v, (tt,)))

                def drain_one():
                    q = projq if projq else outq
                    if q:
                        f, a = q.popleft()
                        f(*a)

                for ch in range(NCH):
                    while projq:
                        f, a = projq.popleft()
                        f(*a)
                    if ch + 1 < NCH:
                        for mt in (0, 4, 1, 5, 2, 6, 3, 7):
                            projq.append((emit_qk, (mt, ch + 1)))
                        for tt in range(4 * (ch + 1), 4 * (ch + 1) + 4):
                            projq.append((emit_v, (tt,)))

                    nkt, w, off = _widths(ch)
                    for hp in range(4):
                        pt = pts[hp % 2]
                        # scores + exp + diagonal masks, spans of 2 k-tiles
                        for s0 in range(0, nkt, 2):
                            s1 = min(s0 + 2, nkt)
                            for par in (0, 1):
                                po = 64 * par
                                sw = off[s1] - off[s0]
                                sp = spool.tile(
                                    [P, 2 * CHW], F32, tag="s",
                                    name=f"sp_{rtag}_{hp}_{ch}_{s0}_{par}",
                                )
                                sl = 0
                                for kt in range(s0, s1):
                                    c0 = CHW - w[kt]
                                    nc.tensor.matmul(
                                        sp[:, sl:sl + w[kt]],
                                        lhsT=qkts[po:po + HD, 4 + hp, kt * P:(kt + 1) * P],
                                        rhs=qkts[po:po + HD, hp, ch * CHW + c0:(ch + 1) * CHW],
                                        start=True,
                                        stop=True,
                                    )
                                    sl += w[kt]
                                nc.scalar.activation(
                                    out=pt[:, par, off[s0]:off[s0] + sw],
                                    in_=sp[:, 0:sw],
                                    func=Exp,
                                    scale=0.125,
                                )
                                for kt in range(s0, s1):
                                    if kt >= 4 * ch:
                                        o = off[kt]
                                        nc.vector.tensor_mul(
                                            out=pt[:, par, o:o + P],
                                            in0=pt[:, par, o:o + P],
                                            in1=tri[:, :],
                                        )
                            drain_one()

                        # AV (q-major, fused denominator), normalize, transpose
                        yt = bpool.tile([P, CHW], BF16, tag="B", name=f"yt_{rtag}_{hp}_{ch}")
                        for par in (0, 1):
                            hl = 2 * hp + par
                            av = apool.tile(
                                [P, 4, HD + 1], F32, tag="av",
                                name=f"av_{rtag}_{hp}_{ch}_{par}",
                            )
                            for qt in range(4):
                                last = 4 * ch + qt
                                for kt in range(last + 1):
                                    c0 = CHW - w[kt]
                                    sl = off[kt] + P * qt - c0
                                    nc.tensor.matmul(
                                        av[:, qt, :],
                                        lhsT=pt[:, par, sl:sl + P],
                                        rhs=vs[:, kt, hl, :],
                                        start=(kt == 0),
                                        stop=(kt == last),
                                    )
                                rec = wpool.tile(
                                    [P, 1], F32, tag="rec",
                                    name=f"rec_{rtag}_{hp}_{ch}_{par}_{qt}",
                                )
                                nc.vector.reciprocal(rec[:, :], av[:, qt, HD:HD + 1])
                                yb = wpool.tile(
                                    [P, HD], BF16, tag="yb",
                                    name=f"yb_{rtag}_{hp}_{ch}_{par}_{qt}",
                                )
                                nc.vector.tensor_scalar_mul(
                                    out=yb[:, :], in0=av[:, qt, 0:HD], scalar1=rec[:, 0:1]
                                )
                                nc.tensor.transpose(
                                    out=yt[64 * par:64 * par + HD, qt * P:(qt + 1) * P],
                                    in_=yb[:, :],
                                    identity=iden[:, :],
                                )
                        nc.vector.tensor_copy(
                            out=yts[:, hp, ch * CHW:(ch + 1) * CHW], in_=yt
                        )
                        drain_one()

                    for tt in range(4 * ch, 4 * ch + 4):
                        for cc in range(2):
                            outq.append((emit_out, (tt, cc)))

                while projq or outq:
                    drain_one()

    nc.finalize()
    return nc


def _core_inputs(x, w_qkv, b_qkv, w_out, core, tri, iden):
    b, g = core // 2, core % 2
    qc = slice(G * g, G * g + G)
    kc = slice(C + G * g, C + G * g + G)
    vc = slice(2 * C + G * g, 2 * C + G * g + G)
    xt = np.ascontiguousarray(x[b].T).astype(bfloat16)
    wqk = np.ascontiguousarray(
        np.concatenate([w_qkv[:, qc], w_qkv[:, kc]], axis=1)
    ).astype(bfloat16)
    bqk = np.ascontiguousarray(
        np.concatenate([b_qkv[qc], b_qkv[kc]]).reshape(8, P).T
    ).astype(np.float32)
    wv = np.ascontiguousarray(w_qkv[:, vc]).astype(bfloat16)
    bv = np.ascontiguousarray(b_qkv[vc].reshape(1, G)).astype(bfloat16)
    wo = np.ascontiguousarray(
        w_out[G * g:G * g + G, :].reshape(4, P, C).transpose(1, 0, 2)
    ).astype(bfloat16)
    return {
        "xt": xt,
        "wqk": wqk,
        "bqk": bqk,
        "wv": wv,
        "bv": bv,
        "wo": wo,
        "tri": tri,
        "iden": iden,
    }


def kernel(x, w_qkv, b_qkv, w_out, b_out):
    global _BUILT, LAST_EXEC_TIME_NS
    x = np.asarray(x, dtype=np.float32)
    w_qkv = np.asarray(w_qkv, dtype=np.float32)
    b_qkv = np.asarray(b_qkv, dtype=np.float32)
    w_out = np.asarray(w_out, dtype=np.float32)
    b_out = np.asarray(b_out, dtype=np.float32)

    if _BUILT is None:
        _BUILT = _build_bass()
    nc = _BUILT

    p = np.arange(P)
    tri = (p[:, None] <= p[None, :]).astype(bfloat16)
    iden = np.eye(P, dtype=bfloat16)
    in_maps = [
        _core_inputs(x, w_qkv, b_qkv, w_out, core, tri, iden) for core in range(8)
    ]
    trace = bool(int(os.environ.get("KERNEL_TRACE", "0")))
    res = run_bass_kernel_spmd(nc, in_maps, list(range(8)), trace=trace)
    LAST_EXEC_TIME_NS = res.exec_time_ns

    out = np.empty((B, T, C), dtype=np.float32)
    for b in range(B):
        out[b] = res.results[2 * b]["out"] + res.results[2 * b + 1]["out"] + b_out
    return out


# revision 13
# speedup vs baseline: 1.4497x; 1.0281x over previous
"""Causal self-attention (B=4, T=2048, C=1024, H=16) on 8 trn2 NeuronCores.

Sharding: core = (batch b, head-group g) with b = core//2, g = core%2.
Each core handles one batch and 8 heads (column-parallel qkv, row-parallel
out_proj).  Cores return partial out-projection results; the host sums the
two head-group partials per batch and adds b_out.

Per-core kernel (all matmuls bf16 with fp32 PSUM accumulate), restructured
for PE occupancy and minimal PE work:
  - streaming startup: x^T loaded in four T-chunks, w_qkv in eight 128-col
    blocks (HWDGE/SP queue), so the first projection matmuls start ~4us in.
  - per q-chunk ch (512 cols): qk projection for that chunk, V projection
    for its four 128-row tiles, then attention for the four head pairs, then
    the chunk's out-projection — out-proj and next-chunk projections are
    emitted as PE "filler" quanta interleaved between score spans so the PE
    stays busy while the Activation engine drains exp() work.
  - scores computed transposed S^T[k, q] with ragged causal widths (the
    k-tiles near the diagonal only compute the surviving q columns).
  - p = exp(0.125 * S^T) on the scalar engine on packed 2-k-tile spans;
    diagonal 128x128 triangles masked by a single precomputed 0/1 mask.
  - AV computed q-major: out[q, d] = sum_k P^T[k,q]^T V[k,d] with a fused
    ones column in V giving the softmax denominator in column 64 — output
    free size is 65 instead of 512, a ~2.3x cut in AV PE time.
  - normalization per-partition (reciprocal + tensor_scalar_mul on DVE),
    then a PE transpose (identity matmul) back to d-major y^T for the
    row-parallel out projection.
"""

import os
from collections import deque

import numpy as np

try:
    import concourse.bass as bass
except ImportError:  # pragma: no cover
    import sys

    sys.path.insert(0, "/opt/trn_rl_repo")
    import concourse.bass as bass

import concourse.mybir as mybir
from concourse.bacc import Bacc
from concourse.bass_utils import run_bass_kernel_spmd
from concourse.tile import TileContext
from ml_dtypes import bfloat16

B, T, C, H = 4, 2048, 1024, 16
HD = 64        # head dim
G = 512        # head-group width: 8 heads * 64
P = 128
NCT = C // P   # contraction tiles over C
NTT = T // P   # 128-row tiles over T
CHW = 512      # q-chunk width
NCH = T // CHW

F32 = mybir.dt.float32
BF16 = mybir.dt.bfloat16

_BUILT = None
LAST_EXEC_TIME_NS = None


def _widths(ch):
    """Per-k-tile ragged score widths and pt-column offsets for chunk ch."""
    nkt = 4 * ch + 4
    w = [CHW - max(0, P * (kt - 4 * ch)) for kt in range(nkt)]
    off = [0]
    for x in w:
        off.append(off[-1] + x)
    return nkt, w, off


PTW = _widths(NCH - 1)[2][-1]  # widest pt row (chunk 3): 7424


def _build_bass(reps=1):
    nc = Bacc("TRN2", target_bir_lowering=False)

    xt_d = nc.dram_tensor("xt", [C, T], BF16, kind="ExternalInput")
    wqk_d = nc.dram_tensor("wqk", [C, 2 * G], BF16, kind="ExternalInput")
    bqk_d = nc.dram_tensor("bqk", [P, 8], F32, kind="ExternalInput")
    wv_d = nc.dram_tensor("wv", [C, G], BF16, kind="ExternalInput")
    bvb_d = nc.dram_tensor("bvb", [P, 8, HD], BF16, kind="ExternalInput")
    wo_d = nc.dram_tensor("wo", [P, 4, C], BF16, kind="ExternalInput")
    tri_d = nc.dram_tensor("tri", [P, P], BF16, kind="ExternalInput")
    iden_d = nc.dram_tensor("iden", [P, P], BF16, kind="ExternalInput")
    out_d = nc.dram_tensor("out", [T, C], F32, kind="ExternalOutput")

    Exp = mybir.ActivationFunctionType.Exp

    with TileContext(nc) as tc:
        with (
            tc.tile_pool(name="const", bufs=1) as cpool,
            tc.tile_pool(name="work", bufs=3) as wpool,
            tc.tile_pool(name="bpool", bufs=2, space="PSUM") as bpool,
            tc.tile_pool(name="spool", bufs=2, space="PSUM") as spool,
            tc.tile_pool(name="apool", bufs=2, space="PSUM") as apool,
        ):
            xts = cpool.tile([P, NCT, T], BF16, tag="xts")
            wqks = cpool.tile([P, NCT, 2 * G], BF16, tag="wqks")
            bqks = cpool.tile([P, 8], F32, tag="bqks")
            wvs = cpool.tile([P, NCT, G], BF16, tag="wvs")
            bvb = cpool.tile([P, 8, HD], BF16, tag="bvb")
            wos = cpool.tile([P, 4, C], BF16, tag="wos")
            tri = cpool.tile([P, P], BF16, tag="tri")
            iden = cpool.tile([P, P], BF16, tag="iden")
            qkts = cpool.tile([P, 8, T], BF16, tag="qkts")
            vs = cpool.tile([P, NTT, 8, HD + 1], BF16, tag="vs")
            yts = cpool.tile([P, 4, T], BF16, tag="yts")
            pts = [
                cpool.tile([P, 2, PTW], BF16, tag=f"pt{i}", name=f"pt{i}")
                for i in range(2)
            ]

            xr = xt_d.rearrange("(a p) t -> p a t", p=P)
            wr = wqk_d.rearrange("(a p) m -> p a m", p=P)
            # DMA issue order ~ readiness order of first consumers.
            nc.sync.dma_start(out=wqks[:, :, 0:P], in_=wr[:, :, 0:P])
            nc.sync.dma_start(out=xts[:, :, 0:CHW], in_=xr[:, :, 0:CHW])
            nc.sync.dma_start(out=bqks[:, :], in_=bqk_d[:, :])
            nc.sync.dma_start(out=wqks[:, :, 4 * P:5 * P], in_=wr[:, :, 4 * P:5 * P])
            nc.sync.dma_start(out=tri[:, :], in_=tri_d[:, :])
            nc.sync.dma_start(out=iden[:, :], in_=iden_d[:, :])
            for mt in (1, 5, 2, 6, 3, 7):
                nc.sync.dma_start(
                    out=wqks[:, :, mt * P:(mt + 1) * P], in_=wr[:, :, mt * P:(mt + 1) * P]
                )
            nc.sync.dma_start(out=wvs[:, :, :], in_=wv_d.rearrange("(a p) m -> p a m", p=P))
            nc.sync.dma_start(out=bvb[:, :, :], in_=bvb_d[:, :, :])
            for c in (1, 2, 3):
                nc.sync.dma_start(
                    out=xts[:, :, c * CHW:(c + 1) * CHW], in_=xr[:, :, c * CHW:(c + 1) * CHW]
                )
            nc.sync.dma_start(out=wos[:, :, :], in_=wo_d[:, :, :])

            for rep in range(reps):
                rtag = f"r{rep}"

                def emit_qk(mt, ch):
                    ps = bpool.tile([P, CHW], F32, tag="B", name=f"qk_{rtag}_{mt}_{ch}")
                    for ct in range(NCT):
                        nc.tensor.matmul(
                            ps,
                            lhsT=wqks[:, ct, mt * P:(mt + 1) * P],
                            rhs=xts[:, ct, ch * CHW:(ch + 1) * CHW],
                            start=(ct == 0),
                            stop=(ct == NCT - 1),
                        )
                    nc.vector.tensor_scalar_add(
                        out=qkts[:, mt, ch * CHW:(ch + 1) * CHW],
                        in0=ps,
                        scalar1=bqks[:, mt:mt + 1],
                    )

                def emit_v(tt):
                    ps = bpool.tile([P, CHW], F32, tag="B", name=f"v_{rtag}_{tt}")
                    for ct in range(NCT):
                        nc.tensor.matmul(
                            ps,
                            lhsT=xts[:, ct, tt * P:(tt + 1) * P],
                            rhs=wvs[:, ct, :],
                            start=(ct == 0),
                            stop=(ct == NCT - 1),
                        )
                    nc.vector.memset(vs[:, tt, :, HD:HD + 1], 1.0)
                    nc.vector.tensor_add(
                        out=vs[:, tt, :, 0:HD],
                        in0=ps.rearrange("p (h d) -> p h d", h=8),
                        in1=bvb[:, :, :],
                    )

                def emit_out(tt, cc):
                    ps = bpool.tile([P, CHW], F32, tag="B", name=f"o_{rtag}_{tt}_{cc}")
                    for j in range(4):
                        nc.tensor.matmul(
                            ps,
                            lhsT=yts[:, j, tt * P:(tt + 1) * P],
                            rhs=wos[:, j, cc * CHW:(cc + 1) * CHW],
                            start=(j == 0),
                            stop=(j == 3),
                        )
                    ev = wpool.tile([P, CHW], F32, tag="ev", name=f"ev_{rtag}_{tt}_{cc}")
                    nc.vector.tensor_copy(out=ev[:, :], in_=ps)
                    nc.sync.dma_start(
                        out=out_d[tt * P:(tt + 1) * P, cc * CHW:(cc + 1) * CHW],
                        in_=ev[:, :],
                    )

                projq = deque()
                outq = deque()
                for mt in (0, 4, 1, 5, 2, 6, 3, 7):
                    projq.append((emit_qk, (mt, 0)))
                for tt in range(4):
                    projq.append((emit_v, (tt,)))

                def drain_one():
                    q = projq if projq else outq
                    if q:
                        f, a = q.popleft()
                        f(*a)

                for ch in range(NCH):
                    while projq:
                        f, a = projq.popleft()
                        f(*a)
                    if ch + 1 < NCH:
                        for mt in (0, 4, 1, 5, 2, 6, 3, 7):
                            projq.append((emit_qk, (mt, ch + 1)))
                        for tt in range(4 * (ch + 1), 4 * (ch + 1) + 4):
                            projq.append((emit_v, (tt,)))

                    nkt, w, off = _widths(ch)
                    nspans = -(-nkt // 2)
                    # evenly pace filler drains across this chunk's drain slots
                    nslots = 4 * (nspans + 1)
                    nquanta = len(projq) + len(outq)
                    drained = [0]
                    slot = [0]

                    def pace():
                        slot[0] += 1
                        want = (nquanta * slot[0] + nslots - 1) // nslots
                        while drained[0] < want and (projq or outq):
                            drain_one()
                            drained[0] += 1

                    for hp in range(4):
                        pt = pts[hp % 2]
                        # scores + exp + diagonal masks, spans of 2 k-tiles
                        for s0 in range(0, nkt, 2):
                            s1 = min(s0 + 2, nkt)
                            for par in (0, 1):
                                po = 64 * par
                                sw = off[s1] - off[s0]
                                sp = spool.tile(
                                    [P, 2 * CHW], F32, tag="s",
                                    name=f"sp_{rtag}_{hp}_{ch}_{s0}_{par}",
                                )
                                sl = 0
                                for kt in range(s0, s1):
                                    c0 = CHW - w[kt]
                                    nc.tensor.matmul(
                                        sp[:, sl:sl + w[kt]],
                                        lhsT=qkts[po:po + HD, 4 + hp, kt * P:(kt + 1) * P],
                                        rhs=qkts[po:po + HD, hp, ch * CHW + c0:(ch + 1) * CHW],
                                        start=True,
                                        stop=True,
                                    )
                                    sl += w[kt]
                                nc.scalar.activation(
                                    out=pt[:, par, off[s0]:off[s0] + sw],
                                    in_=sp[:, 0:sw],
                                    func=Exp,
                                    scale=0.125,
                                )
                                for kt in range(s0, s1):
                                    if kt >= 4 * ch:
                                        o = off[kt]
                                        nc.vector.tensor_mul(
                                            out=pt[:, par, o:o + P],
                                            in0=pt[:, par, o:o + P],
                                            in1=tri[:, :],
                                        )
                            pace()

                        # AV (q-major, fused denominator): all 8 chains first,
                        # then all 8 transposes, so the PE never waits on the
                        # DVE reciprocal+scale chain of the tile it just made.
                        yt = bpool.tile([P, CHW], BF16, tag="B", name=f"yt_{rtag}_{hp}_{ch}")
                        ybs = {}
                        for par in (0, 1):
                            hl = 2 * hp + par
                            av = apool.tile(
                                [P, 4, HD + 1], F32, tag="av",
                                name=f"av_{rtag}_{hp}_{ch}_{par}",
                            )
                            for qt in range(4):
                                last = 4 * ch + qt
                                for kt in range(last + 1):
                                    c0 = CHW - w[kt]
                                    sl = off[kt] + P * qt - c0
                                    nc.tensor.matmul(
                                        av[:, qt, :],
                                        lhsT=pt[:, par, sl:sl + P],
                                        rhs=vs[:, kt, hl, :],
                                        start=(kt == 0),
                                        stop=(kt == last),
                                    )
                                rec = wpool.tile(
                                    [P, 1], F32, tag="rec", bufs=4,
                                    name=f"rec_{rtag}_{hp}_{ch}_{par}_{qt}",
                                )
                                nc.vector.reciprocal(rec[:, :], av[:, qt, HD:HD + 1])
                                yb = wpool.tile(
                                    [P, HD], BF16, tag="yb", bufs=10,
                                    name=f"yb_{rtag}_{hp}_{ch}_{par}_{qt}",
                                )
                                nc.vector.tensor_scalar_mul(
                                    out=yb[:, :], in0=av[:, qt, 0:HD], scalar1=rec[:, 0:1]
                                )
                                ybs[(par, qt)] = yb
                        for par in (0, 1):
                            for qt in range(4):
                                nc.tensor.transpose(
                                    out=yt[64 * par:64 * par + HD, qt * P:(qt + 1) * P],
                                    in_=ybs[(par, qt)][:, :],
                                    identity=iden[:, :],
                                )
                        nc.vector.tensor_copy(
                            out=yts[:, hp, ch * CHW:(ch + 1) * CHW], in_=yt
                        )
                        pace()

                    for tt in range(4 * ch, 4 * ch + 4):
                        for cc in range(2):
                            outq.append((emit_out, (tt, cc)))

                while projq or outq:
                    drain_one()

    nc.finalize()
    return nc


def _core_inputs(x, w_qkv, b_qkv, w_out, core, tri, iden):
    b, g = core // 2, core % 2
    qc = slice(G * g, G * g + G)
    kc = slice(C + G * g, C + G * g + G)
    vc = slice(2 * C + G * g, 2 * C + G * g + G)
    xt = np.ascontiguousarray(x[b].T).astype(bfloat16)
    wqk = np.ascontiguousarray(
        np.concatenate([w_qkv[:, qc], w_qkv[:, kc]], axis=1)
    ).astype(bfloat16)
    bqk = np.ascontiguousarray(
        np.concatenate([b_qkv[qc], b_qkv[kc]]).reshape(8, P).T
    ).astype(np.float32)
    wv = np.ascontiguousarray(w_qkv[:, vc]).astype(bfloat16)
    bvb = np.ascontiguousarray(
        np.broadcast_to(b_qkv[vc].reshape(1, 8, HD), (P, 8, HD))
    ).astype(bfloat16)
    wo = np.ascontiguousarray(
        w_out[G * g:G * g + G, :].reshape(4, P, C).transpose(1, 0, 2)
    ).astype(bfloat16)
    return {
        "xt": xt,
        "wqk": wqk,
        "bqk": bqk,
        "wv": wv,
        "bvb": bvb,
        "wo": wo,
        "tri": tri,
        "iden": iden,
    }


def kernel(x, w_qkv, b_qkv, w_out, b_out):
    global _BUILT, LAST_EXEC_TIME_NS
    x = np.asarray(x, dtype=np.float32)
    w_qkv = np.asarray(w_qkv, dtype=np.float32)
    b_qkv = np.asarray(b_qkv, dtype=np.float32)
    w_out = np.asarray(w_out, dtype=np.float32)
    b_out = np.asarray(b_out, dtype=np.float32)

    if _BUILT is None:
        _BUILT = _build_bass()
    nc = _BUILT

    p = np.arange(P)
    tri = (p[:, None] <= p[None, :]).astype(bfloat16)
    iden = np.eye(P, dtype=bfloat16)
    in_maps = [
        _core_inputs(x, w_qkv, b_qkv, w_out, core, tri, iden) for core in range(8)
    ]
    trace = bool(int(os.environ.get("KERNEL_TRACE", "0")))
    res = run_bass_kernel_spmd(nc, in_maps, list(range(8)), trace=trace)
    LAST_EXEC_TIME_NS = res.exec_time_ns

    out = np.empty((B, T, C), dtype=np.float32)
    for b in range(B):
        out[b] = res.results[2 * b]["out"] + res.results[2 * b + 1]["out"] + b_out
    return out


# revision 22
# speedup vs baseline: 1.4892x; 1.0273x over previous
"""Causal self-attention (B=4, T=2048, C=1024, H=16) on 8 trn2 NeuronCores.

Sharding: core = (batch b, head-group g) with b = core//2, g = core%2.
Each core handles one batch and 8 heads (column-parallel qkv, row-parallel
out_proj).  Cores return partial out-projection results; the host sums the
two head-group partials per batch and adds b_out.

Per-core kernel (all matmuls bf16 with fp32 PSUM accumulate), restructured
for PE occupancy and minimal PE work:
  - streaming startup: x^T loaded in four T-chunks, w_qkv in eight 128-col
    blocks (HWDGE/SP queue), so the first projection matmuls start ~4us in.
  - per q-chunk ch (512 cols): qk projection for that chunk, V projection
    for its four 128-row tiles, then attention for the four head pairs, then
    the chunk's out-projection — out-proj and next-chunk projections are
    emitted as PE "filler" quanta interleaved between score spans so the PE
    stays busy while the Activation engine drains exp() work.
  - scores computed transposed S^T[k, q] with ragged causal widths (the
    k-tiles near the diagonal only compute the surviving q columns).
  - p = exp(0.125 * S^T) on the scalar engine on packed 2-k-tile spans;
    diagonal 128x128 triangles masked by a single precomputed 0/1 mask.
  - AV computed q-major: out[q, d] = sum_k P^T[k,q]^T V[k,d] with a fused
    ones column in V giving the softmax denominator in column 64 — output
    free size is 65 instead of 512, a ~2.3x cut in AV PE time.
  - normalization per-partition (reciprocal + tensor_scalar_mul on DVE),
    then a PE transpose (identity matmul) back to d-major y^T for the
    row-parallel out projection.
"""

import os
from collections import deque

import numpy as np

try:
    import concourse.bass as bass
except ImportError:  # pragma: no cover
    import sys

    sys.path.insert(0, "/opt/trn_rl_repo")
    import concourse.bass as bass

import concourse.mybir as mybir
from concourse.bacc import Bacc
from concourse.bass_utils import run_bass_kernel_spmd
from concourse.tile import TileContext
from ml_dtypes import bfloat16

B, T, C, H = 4, 2048, 1024, 16
HD = 64        # head dim
G = 512        # head-group width: 8 heads * 64
P = 128
NCT = C // P   # contraction tiles over C
NTT = T // P   # 128-row tiles over T
CHW = 512      # q-chunk width
NCH = T // CHW

F32 = mybir.dt.float32
BF16 = mybir.dt.bfloat16

_BUILT = None
LAST_EXEC_TIME_NS = None


def _widths(ch):
    """Per-k-tile ragged score widths and pt-column offsets for chunk ch."""
    nkt = 4 * ch + 4
    w = [CHW - max(0, P * (kt - 4 * ch)) for kt in range(nkt)]
    off = [0]
    for x in w:
        off.append(off[-1] + x)
    return nkt, w, off


PTW = _widths(NCH - 1)[2][-1]  # widest pt row (chunk 3): 7424


def _build_bass(reps=1):
    nc = Bacc("TRN2", target_bir_lowering=False)

    xt_d = nc.dram_tensor("xt", [C, T], BF16, kind="ExternalInput")
    wqk_d = nc.dram_tensor("wqk", [C, 2 * G], BF16, kind="ExternalInput")
    bqk_d = nc.dram_tensor("bqk", [P, 8], F32, kind="ExternalInput")
    wv_d = nc.dram_tensor("wv", [C, G], BF16, kind="ExternalInput")
    bvb_d = nc.dram_tensor("bvb", [P, 8, HD], BF16, kind="ExternalInput")
    wo_d = nc.dram_tensor("wo", [P, 4, C], BF16, kind="ExternalInput")
    tri_d = nc.dram_tensor("tri", [P, P], BF16, kind="ExternalInput")
    iden_d = nc.dram_tensor("iden", [P, P], BF16, kind="ExternalInput")
    out_d = nc.dram_tensor("out", [T, C], F32, kind="ExternalOutput")

    Exp = mybir.ActivationFunctionType.Exp

    with TileContext(nc) as tc:
        with (
            tc.tile_pool(name="const", bufs=1) as cpool,
            tc.tile_pool(name="work", bufs=3) as wpool,
            tc.tile_pool(name="bpool", bufs=2, space="PSUM") as bpool,
            tc.tile_pool(name="spool", bufs=2, space="PSUM") as spool,
            tc.tile_pool(name="apool", bufs=2, space="PSUM") as apool,
        ):
            xts = cpool.tile([P, NCT, T], BF16, tag="xts")
            wqks = cpool.tile([P, NCT, 2 * G], BF16, tag="wqks")
            bqks = cpool.tile([P, 8], F32, tag="bqks")
            wvs = cpool.tile([P, NCT, G], BF16, tag="wvs")
            bvb = cpool.tile([P, 8, HD], BF16, tag="bvb")
            wos = cpool.tile([P, 4, C], BF16, tag="wos")
            tri = cpool.tile([P, P], BF16, tag="tri")
            iden = cpool.tile([P, P], BF16, tag="iden")
            qkts = cpool.tile([P, 8, T], BF16, tag="qkts")
            vs = cpool.tile([P, NTT, 8, HD + 1], BF16, tag="vs")
            yts = cpool.tile([P, 4, T], BF16, tag="yts")
            pts = [
                cpool.tile([P, 2, PTW], BF16, tag=f"pt{i}", name=f"pt{i}")
                for i in range(2)
            ]

            xr = xt_d.rearrange("(a p) t -> p a t", p=P)
            wr = wqk_d.rearrange("(a p) m -> p a m", p=P)
            # DMA issue order ~ readiness order of first consumers.  The first
            # x chunk is split per contraction tile so the first qk projection
            # matmuls can start as soon as tile 0 lands.
            nc.sync.dma_start(out=bqks[:, :], in_=bqk_d[:, :])
            nc.sync.dma_start(out=wqks[:, :, 0:P], in_=wr[:, :, 0:P])
            for ct in range(2):
                nc.sync.dma_start(
                    out=xts[:, ct, 0:CHW], in_=xr[:, ct, 0:CHW]
                )
            nc.sync.dma_start(out=wqks[:, :, 4 * P:5 * P], in_=wr[:, :, 4 * P:5 * P])
            for ct in range(2, NCT):
                nc.sync.dma_start(
                    out=xts[:, ct, 0:CHW], in_=xr[:, ct, 0:CHW]
                )
            nc.sync.dma_start(out=tri[:, :], in_=tri_d[:, :])
            nc.sync.dma_start(out=iden[:, :], in_=iden_d[:, :])
            for mt in (1, 5, 2, 6, 3, 7):
                nc.sync.dma_start(
                    out=wqks[:, :, mt * P:(mt + 1) * P], in_=wr[:, :, mt * P:(mt + 1) * P]
                )
            nc.sync.dma_start(out=wvs[:, :, :], in_=wv_d.rearrange("(a p) m -> p a m", p=P))
            nc.sync.dma_start(out=bvb[:, :, :], in_=bvb_d[:, :, :])
            for c in (1, 2, 3):
                nc.sync.dma_start(
                    out=xts[:, :, c * CHW:(c + 1) * CHW], in_=xr[:, :, c * CHW:(c + 1) * CHW]
                )
            nc.sync.dma_start(out=wos[:, :, :], in_=wo_d[:, :, :])

            for rep in range(reps):
                rtag = f"r{rep}"

                def emit_qk(mt, ch):
                    ps = bpool.tile([P, CHW], F32, tag="B", name=f"qk_{rtag}_{mt}_{ch}")
                    for ct in range(NCT):
                        nc.tensor.matmul(
                            ps,
                            lhsT=wqks[:, ct, mt * P:(mt + 1) * P],
                            rhs=xts[:, ct, ch * CHW:(ch + 1) * CHW],
                            start=(ct == 0),
                            stop=(ct == NCT - 1),
                        )
                    nc.vector.tensor_scalar_add(
                        out=qkts[:, mt, ch * CHW:(ch + 1) * CHW],
                        in0=ps,
                        scalar1=bqks[:, mt:mt + 1],
                    )

                def emit_v(tt):
                    ps = bpool.tile([P, CHW], F32, tag="B", name=f"v_{rtag}_{tt}")
                    for ct in range(NCT):
                        nc.tensor.matmul(
                            ps,
                            lhsT=xts[:, ct, tt * P:(tt + 1) * P],
                            rhs=wvs[:, ct, :],
                            start=(ct == 0),
                            stop=(ct == NCT - 1),
                        )
                    nc.vector.memset(vs[:, tt, :, HD:HD + 1], 1.0)
                    nc.vector.tensor_add(
                        out=vs[:, tt, :, 0:HD],
                        in0=ps.rearrange("p (h d) -> p h d", h=8),
                        in1=bvb[:, :, :],
                    )

                def emit_out(tt, cc, on_act=False):
                    ps = bpool.tile([P, CHW], F32, tag="B", name=f"o_{rtag}_{tt}_{cc}")
                    for j in range(4):
                        nc.tensor.matmul(
                            ps,
                            lhsT=yts[:, j, tt * P:(tt + 1) * P],
                            rhs=wos[:, j, cc * CHW:(cc + 1) * CHW],
                            start=(j == 0),
                            stop=(j == 3),
                        )
                    ev = wpool.tile([P, CHW], F32, tag="ev", name=f"ev_{rtag}_{tt}_{cc}")
                    if on_act:
                        nc.scalar.activation(
                            out=ev[:, :], in_=ps,
                            func=mybir.ActivationFunctionType.Copy,
                        )
                    else:
                        nc.vector.tensor_copy(out=ev[:, :], in_=ps)
                    nc.sync.dma_start(
                        out=out_d[tt * P:(tt + 1) * P, cc * CHW:(cc + 1) * CHW],
                        in_=ev[:, :],
                    )

                projq = deque()
                outq = deque()
                for mt in (0, 4, 1, 5, 2, 6, 3, 7):
                    projq.append((emit_qk, (mt, 0)))
                for tt in range(4):
                    projq.append((emit_v, (tt,)))

                for ch in range(NCH):
                    while projq:
                        f, a = projq.popleft()
                        f(*a)
                    if ch + 1 < NCH:
                        for mt in (0, 4, 1, 5, 2, 6, 3, 7):
                            projq.append((emit_qk, (mt, ch + 1)))
                        for tt in range(4 * (ch + 1), 4 * (ch + 1) + 4):
                            projq.append((emit_v, (tt,)))

                    # out-proj quanta are the only filler available during the
                    # last chunk (no next-chunk projections), which is also
                    # where the Activation engine's exp backlog is largest —
                    # hold back a reserve of them on earlier chunks.
                    keep = 8 if ch + 1 < NCH else 0

                    def drain_one():
                        if projq:
                            f, a = projq.popleft()
                            f(*a)
                            return True
                        if len(outq) > keep:
                            f, a = outq.popleft()
                            f(*a)
                            return True
                        return False

                    nkt, w, off = _widths(ch)
                    nspans = -(-nkt // 2)
                    # evenly pace filler drains across this chunk's drain slots
                    nslots = 4 * (nspans + 1)
                    nquanta = len(projq) + max(0, len(outq) - keep)
                    drained = [0]
                    slot = [0]

                    def pace():
                        slot[0] += 1
                        want = (nquanta * slot[0] + nslots - 1) // nslots
                        while drained[0] < want and drain_one():
                            drained[0] += 1

                    for hp in range(4):
                        pt = pts[hp % 2]
                        # scores + exp + diagonal masks, spans of 2 k-tiles
                        for s0 in range(0, nkt, 2):
                            s1 = min(s0 + 2, nkt)
                            for par in (0, 1):
                                po = 64 * par
                                sw = off[s1] - off[s0]
                                sp = spool.tile(
                                    [P, 2 * CHW], F32, tag="s",
                                    name=f"sp_{rtag}_{hp}_{ch}_{s0}_{par}",
                                )
                                sl = 0
                                for kt in range(s0, s1):
                                    c0 = CHW - w[kt]
                                    nc.tensor.matmul(
                                        sp[:, sl:sl + w[kt]],
                                        lhsT=qkts[po:po + HD, 4 + hp, kt * P:(kt + 1) * P],
                                        rhs=qkts[po:po + HD, hp, ch * CHW + c0:(ch + 1) * CHW],
                                        start=True,
                                        stop=True,
                                    )
                                    sl += w[kt]
                                nc.scalar.activation(
                                    out=pt[:, par, off[s0]:off[s0] + sw],
                                    in_=sp[:, 0:sw],
                                    func=Exp,
                                    scale=0.125,
                                )
                                for kt in range(s0, s1):
                                    if kt >= 4 * ch:
                                        o = off[kt]
                                        nc.vector.tensor_mul(
                                            out=pt[:, par, o:o + P],
                                            in0=pt[:, par, o:o + P],
                                            in1=tri[:, :],
                                        )
                            pace()

                        # AV (q-major, fused denominator): all 8 chains first,
                        # then all 8 transposes, so the PE never waits on the
                        # DVE reciprocal+scale chain of the tile it just made.
                        yt = bpool.tile([P, CHW], BF16, tag="B", name=f"yt_{rtag}_{hp}_{ch}")
                        ybs = {}
                        for par in (0, 1):
                            hl = 2 * hp + par
                            av = apool.tile(
                                [P, 4, HD + 1], F32, tag="av",
                                name=f"av_{rtag}_{hp}_{ch}_{par}",
                            )
                            for qt in range(4):
                                last = 4 * ch + qt
                                for kt in range(last + 1):
                                    c0 = CHW - w[kt]
                                    sl = off[kt] + P * qt - c0
                                    nc.tensor.matmul(
                                        av[:, qt, :],
                                        lhsT=pt[:, par, sl:sl + P],
                                        rhs=vs[:, kt, hl, :],
                                        start=(kt == 0),
                                        stop=(kt == last),
                                    )
                                rec = wpool.tile(
                                    [P, 1], F32, tag="rec", bufs=4,
                                    name=f"rec_{rtag}_{hp}_{ch}_{par}_{qt}",
                                )
                                nc.vector.reciprocal(rec[:, :], av[:, qt, HD:HD + 1])
                                yb = wpool.tile(
                                    [P, HD], BF16, tag="yb", bufs=10,
                                    name=f"yb_{rtag}_{hp}_{ch}_{par}_{qt}",
                                )
                                nc.vector.tensor_scalar_mul(
                                    out=yb[:, :], in0=av[:, qt, 0:HD], scalar1=rec[:, 0:1]
                                )
                                ybs[(par, qt)] = yb
                        for par in (0, 1):
                            for qt in range(4):
                                nc.tensor.transpose(
                                    out=yt[64 * par:64 * par + HD, qt * P:(qt + 1) * P],
                                    in_=ybs[(par, qt)][:, :],
                                    identity=iden[:, :],
                                )
                        nc.vector.tensor_copy(
                            out=yts[:, hp, ch * CHW:(ch + 1) * CHW], in_=yt
                        )
                        pace()

                    for tt in range(4 * ch, 4 * ch + 4):
                        for cc in range(2):
                            outq.append((emit_out, (tt, cc)))

                while projq:
                    f, a = projq.popleft()
                    f(*a)
                # final out-proj groups: evict on the now-idle Act engine
                while outq:
                    f, (tt, cc) = outq.popleft()
                    f(tt, cc, True)

    nc.finalize()
    return nc


def _core_inputs(x, w_qkv, b_qkv, w_out, core, tri, iden):
    b, g = core // 2, core % 2
    qc = slice(G * g, G * g + G)
    kc = slice(C + G * g, C + G * g + G)
    vc = slice(2 * C + G * g, 2 * C + G * g + G)
    xt = np.ascontiguousarray(x[b].T).astype(bfloat16)
    wqk = np.ascontiguousarray(
        np.concatenate([w_qkv[:, qc], w_qkv[:, kc]], axis=1)
    ).astype(bfloat16)
    bqk = np.ascontiguousarray(
        np.concatenate([b_qkv[qc], b_qkv[kc]]).reshape(8, P).T
    ).astype(np.float32)
    wv = np.ascontiguousarray(w_qkv[:, vc]).astype(bfloat16)
    bvb = np.ascontiguousarray(
        np.broadcast_to(b_qkv[vc].reshape(1, 8, HD), (P, 8, HD))
    ).astype(bfloat16)
    wo = np.ascontiguousarray(
        w_out[G * g:G * g + G, :].reshape(4, P, C).transpose(1, 0, 2)
    ).astype(bfloat16)
    return {
        "xt": xt,
        "wqk": wqk,
        "bqk": bqk,
        "wv": wv,
        "bvb": bvb,
        "wo": wo,
        "tri": tri,
        "iden": iden,
    }


def kernel(x, w_qkv, b_qkv, w_out, b_out):
    global _BUILT, LAST_EXEC_TIME_NS
    x = np.asarray(x, dtype=np.float32)
    w_qkv = np.asarray(w_qkv, dtype=np.float32)
    b_qkv = np.asarray(b_qkv, dtype=np.float32)
    w_out = np.asarray(w_out, dtype=np.float32)
    b_out = np.asarray(b_out, dtype=np.float32)

    if _BUILT is None:
        _BUILT = _build_bass()
    nc = _BUILT

    p = np.arange(P)
    tri = (p[:, None] <= p[None, :]).astype(bfloat16)
    iden = np.eye(P, dtype=bfloat16)
    in_maps = [
        _core_inputs(x, w_qkv, b_qkv, w_out, core, tri, iden) for core in range(8)
    ]
    trace = bool(int(os.environ.get("KERNEL_TRACE", "0")))
    res = run_bass_kernel_spmd(nc, in_maps, list(range(8)), trace=trace)
    LAST_EXEC_TIME_NS = res.exec_time_ns

    out = np.empty((B, T, C), dtype=np.float32)
    for b in range(B):
        out[b] = res.results[2 * b]["out"] + res.results[2 * b + 1]["out"] + b_out
    return out


# revision 26
# speedup vs baseline: 1.5148x; 1.0172x over previous
"""Causal self-attention (B=4, T=2048, C=1024, H=16) on 8 trn2 NeuronCores.

Sharding: core = (batch b, head-group g) with b = core//2, g = core%2.
Each core handles one batch and 8 heads (column-parallel qkv, row-parallel
out_proj).  Cores return partial out-projection results; the host sums the
two head-group partials per batch and adds b_out.

Per-core kernel (all matmuls bf16 with fp32 PSUM accumulate), restructured
for PE occupancy and minimal PE work:
  - streaming startup: x^T loaded in four T-chunks, w_qkv in eight 128-col
    blocks (HWDGE/SP queue), so the first projection matmuls start ~4us in.
  - per q-chunk ch (512 cols): qk projection for that chunk, V projection
    for its four 128-row tiles, then attention for the four head pairs, then
    the chunk's out-projection — out-proj and next-chunk projections are
    emitted as PE "filler" quanta interleaved between score spans so the PE
    stays busy while the Activation engine drains exp() work.
  - scores computed transposed S^T[k, q] with ragged causal widths (the
    k-tiles near the diagonal only compute the surviving q columns).
  - p = exp(0.125 * S^T) on the scalar engine on packed 2-k-tile spans;
    diagonal 128x128 triangles masked by a single precomputed 0/1 mask.
  - AV computed q-major: out[q, d] = sum_k P^T[k,q]^T V[k,d] with a fused
    ones column in V giving the softmax denominator in column 64 — output
    free size is 65 instead of 512, a ~2.3x cut in AV PE time.
  - normalization per-partition (reciprocal + tensor_scalar_mul on DVE),
    then a PE transpose (identity matmul) back to d-major y^T for the
    row-parallel out projection.
"""

import os
from collections import deque

import numpy as np

try:
    import concourse.bass as bass
except ImportError:  # pragma: no cover
    import sys

    sys.path.insert(0, "/opt/trn_rl_repo")
    import concourse.bass as bass

import concourse.mybir as mybir
from concourse.bacc import Bacc
from concourse.bass_utils import run_bass_kernel_spmd
from concourse.tile import TileContext
from ml_dtypes import bfloat16

B, T, C, H = 4, 2048, 1024, 16
HD = 64        # head dim
G = 512        # head-group width: 8 heads * 64
P = 128
NCT = C // P   # contraction tiles over C
NTT = T // P   # 128-row tiles over T
CHW = 512      # q-chunk width
NCH = T // CHW

F32 = mybir.dt.float32
BF16 = mybir.dt.bfloat16

_BUILT = None
LAST_EXEC_TIME_NS = None


def _widths(ch):
    """Per-k-tile ragged score widths and pt-column offsets for chunk ch."""
    nkt = 4 * ch + 4
    w = [CHW - max(0, P * (kt - 4 * ch)) for kt in range(nkt)]
    off = [0]
    for x in w:
        off.append(off[-1] + x)
    return nkt, w, off


PTW = _widths(NCH - 1)[2][-1]  # widest pt row (chunk 3): 7424


def _build_bass(reps=1):
    nc = Bacc("TRN2", target_bir_lowering=False)

    xt_d = nc.dram_tensor("xt", [C, T], BF16, kind="ExternalInput")
    wqk_d = nc.dram_tensor("wqk", [C, 2 * G], BF16, kind="ExternalInput")
    bqk_d = nc.dram_tensor("bqk", [P, 8], F32, kind="ExternalInput")
    wv_d = nc.dram_tensor("wv", [C, G], BF16, kind="ExternalInput")
    bvb_d = nc.dram_tensor("bvb", [P, 8, HD], BF16, kind="ExternalInput")
    wo_d = nc.dram_tensor("wo", [P, 4, C], BF16, kind="ExternalInput")
    tri_d = nc.dram_tensor("tri", [P, P], BF16, kind="ExternalInput")
    iden_d = nc.dram_tensor("iden", [P, P], BF16, kind="ExternalInput")
    out_d = nc.dram_tensor("out", [T, C], BF16, kind="ExternalOutput")

    Exp = mybir.ActivationFunctionType.Exp

    with TileContext(nc) as tc:
        with (
            tc.tile_pool(name="const", bufs=1) as cpool,
            tc.tile_pool(name="work", bufs=3) as wpool,
            tc.tile_pool(name="bpool", bufs=2, space="PSUM") as bpool,
            tc.tile_pool(name="spool", bufs=2, space="PSUM") as spool,
            tc.tile_pool(name="apool", bufs=2, space="PSUM") as apool,
        ):
            xts = cpool.tile([P, NCT, T], BF16, tag="xts")
            wqks = cpool.tile([P, NCT, 2 * G], BF16, tag="wqks")
            bqks = cpool.tile([P, 8], F32, tag="bqks")
            wvs = cpool.tile([P, NCT, G], BF16, tag="wvs")
            bvb = cpool.tile([P, 8, HD], BF16, tag="bvb")
            wos = cpool.tile([P, 4, C], BF16, tag="wos")
            tri = cpool.tile([P, P], BF16, tag="tri")
            iden = cpool.tile([P, P], BF16, tag="iden")
            qkts = cpool.tile([P, 8, T], BF16, tag="qkts")
            vs = cpool.tile([P, NTT, 8, HD + 1], BF16, tag="vs")
            yts = cpool.tile([P, 4, T], BF16, tag="yts")
            pts = [
                cpool.tile([P, 2, PTW], BF16, tag=f"pt{i}", name=f"pt{i}")
                for i in range(2)
            ]

            xr = xt_d.rearrange("(a p) t -> p a t", p=P)
            wr = wqk_d.rearrange("(a p) m -> p a m", p=P)
            # DMA issue order ~ readiness order of first consumers.  The first
            # x chunk is split per contraction tile so the first qk projection
            # matmuls can start as soon as tile 0 lands.
            nc.sync.dma_start(out=bqks[:, :], in_=bqk_d[:, :])
            nc.sync.dma_start(out=wqks[:, :, 0:P], in_=wr[:, :, 0:P])
            for ct in range(2):
                nc.sync.dma_start(
                    out=xts[:, ct, 0:CHW], in_=xr[:, ct, 0:CHW]
                )
            nc.sync.dma_start(out=wqks[:, :, 4 * P:5 * P], in_=wr[:, :, 4 * P:5 * P])
            for ct in range(2, NCT):
                nc.sync.dma_start(
                    out=xts[:, ct, 0:CHW], in_=xr[:, ct, 0:CHW]
                )
            nc.sync.dma_start(out=tri[:, :], in_=tri_d[:, :])
            nc.sync.dma_start(out=iden[:, :], in_=iden_d[:, :])
            for mt in (1, 5, 2, 6, 3, 7):
                nc.sync.dma_start(
                    out=wqks[:, :, mt * P:(mt + 1) * P], in_=wr[:, :, mt * P:(mt + 1) * P]
                )
            nc.sync.dma_start(out=wvs[:, :, :], in_=wv_d.rearrange("(a p) m -> p a m", p=P))
            nc.sync.dma_start(out=bvb[:, :, :], in_=bvb_d[:, :, :])
            for c in (1, 2, 3):
                nc.sync.dma_start(
                    out=xts[:, :, c * CHW:(c + 1) * CHW], in_=xr[:, :, c * CHW:(c + 1) * CHW]
                )
            nc.sync.dma_start(out=wos[:, :, :], in_=wo_d[:, :, :])

            for rep in range(reps):
                rtag = f"r{rep}"

                def _qk_evict(ps, mt, ch, on_act):
                    if on_act:
                        nc.scalar.activation(
                            out=qkts[:, mt, ch * CHW:(ch + 1) * CHW],
                            in_=ps,
                            func=mybir.ActivationFunctionType.Identity,
                            bias=bqks[:, mt:mt + 1],
                        )
                    else:
                        nc.vector.tensor_scalar_add(
                            out=qkts[:, mt, ch * CHW:(ch + 1) * CHW],
                            in0=ps,
                            scalar1=bqks[:, mt:mt + 1],
                        )

                def emit_qk(mt, ch):
                    ps = bpool.tile([P, CHW], F32, tag="B", name=f"qk_{rtag}_{mt}_{ch}")
                    for ct in range(NCT):
                        nc.tensor.matmul(
                            ps,
                            lhsT=wqks[:, ct, mt * P:(mt + 1) * P],
                            rhs=xts[:, ct, ch * CHW:(ch + 1) * CHW],
                            start=(ct == 0),
                            stop=(ct == NCT - 1),
                        )
                    _qk_evict(ps, mt, ch, ch == 0)

                def emit_qk2(mta, mtb, ch):
                    # two qk blocks with interleaved contraction matmuls, so
                    # the second block doesn't wait out the first's x-tile DMAs
                    psa = bpool.tile([P, CHW], F32, tag="B", name=f"qk_{rtag}_{mta}_{ch}")
                    psb = bpool.tile([P, CHW], F32, tag="B", name=f"qk_{rtag}_{mtb}_{ch}")
                    for ct in range(NCT):
                        for mt, ps in ((mta, psa), (mtb, psb)):
                            nc.tensor.matmul(
                                ps,
                                lhsT=wqks[:, ct, mt * P:(mt + 1) * P],
                                rhs=xts[:, ct, ch * CHW:(ch + 1) * CHW],
                                start=(ct == 0),
                                stop=(ct == NCT - 1),
                            )
                    _qk_evict(psa, mta, ch, True)
                    _qk_evict(psb, mtb, ch, ch == 0)

                def emit_v(tt):
                    ps = bpool.tile([P, CHW], F32, tag="B", name=f"v_{rtag}_{tt}")
                    for ct in range(NCT):
                        nc.tensor.matmul(
                            ps,
                            lhsT=xts[:, ct, tt * P:(tt + 1) * P],
                            rhs=wvs[:, ct, :],
                            start=(ct == 0),
                            stop=(ct == NCT - 1),
                        )
                    nc.vector.memset(vs[:, tt, :, HD:HD + 1], 1.0)
                    nc.vector.tensor_add(
                        out=vs[:, tt, :, 0:HD],
                        in0=ps.rearrange("p (h d) -> p h d", h=8),
                        in1=bvb[:, :, :],
                    )

                def emit_out(tt, cc, on_act=False):
                    ps = bpool.tile([P, CHW], F32, tag="B", name=f"o_{rtag}_{tt}_{cc}")
                    for j in range(4):
                        nc.tensor.matmul(
                            ps,
                            lhsT=yts[:, j, tt * P:(tt + 1) * P],
                            rhs=wos[:, j, cc * CHW:(cc + 1) * CHW],
                            start=(j == 0),
                            stop=(j == 3),
                        )
                    ev = wpool.tile([P, CHW], BF16, tag="ev", name=f"ev_{rtag}_{tt}_{cc}")
                    if on_act:
                        nc.scalar.activation(
                            out=ev[:, :], in_=ps,
                            func=mybir.ActivationFunctionType.Copy,
                        )
                    else:
                        nc.vector.tensor_copy(out=ev[:, :], in_=ps)
                    nc.sync.dma_start(
                        out=out_d[tt * P:(tt + 1) * P, cc * CHW:(cc + 1) * CHW],
                        in_=ev[:, :],
                    )

                projq = deque()
                outq = deque()
                projq.append((emit_qk2, (0, 4, 0)))
                for mt in (1, 5, 2, 6, 3, 7):
                    projq.append((emit_qk, (mt, 0)))
                for tt in range(4):
                    projq.append((emit_v, (tt,)))

                for ch in range(NCH):
                    while projq:
                        f, a = projq.popleft()
                        f(*a)
                    if ch + 1 < NCH:
                        for mt in (0, 4, 1, 5, 2, 6, 3, 7):
                            projq.append((emit_qk, (mt, ch + 1)))
                        for tt in range(4 * (ch + 1), 4 * (ch + 1) + 4):
                            projq.append((emit_v, (tt,)))

                    # out-proj quanta are the only filler available during the
                    # last chunk (no next-chunk projections), which is also
                    # where the Activation engine's exp backlog is largest —
                    # hold back a reserve of them on earlier chunks.
                    keep = 8 * ch if ch + 1 < NCH else 0

                    def drain_one():
                        if projq:
                            f, a = projq.popleft()
                            f(*a)
                            return True
                        if len(outq) > keep:
                            f, a = outq.popleft()
                            f(*a)
                            return True
                        return False

                    nkt, w, off = _widths(ch)
                    nspans = -(-nkt // 2)
                    # pace filler drains across this chunk's drain slots;
                    # the slots around the AV chains (where the PE waits on
                    # the exp backlog) get extra weight
                    nslots = 4 * (nspans + 7)
                    nquanta = len(projq) + max(0, len(outq) - keep)
                    drained = [0]
                    slot = [0]

                    def pace(wt=1):
                        slot[0] += wt
                        want = (nquanta * slot[0] + nslots - 1) // nslots
                        while drained[0] < want and drain_one():
                            drained[0] += 1

                    for hp in range(4):
                        pt = pts[hp % 2]
                        # scores + exp + diagonal masks, spans of 2 k-tiles
                        for s0 in range(0, nkt, 2):
                            s1 = min(s0 + 2, nkt)
                            for par in (0, 1):
                                po = 64 * par
                                sw = off[s1] - off[s0]
                                sp = spool.tile(
                                    [P, 2 * CHW], F32, tag="s",
                                    name=f"sp_{rtag}_{hp}_{ch}_{s0}_{par}",
                                )
                                sl = 0
                                for kt in range(s0, s1):
                                    c0 = CHW - w[kt]
                                    nc.tensor.matmul(
                                        sp[:, sl:sl + w[kt]],
                                        lhsT=qkts[po:po + HD, 4 + hp, kt * P:(kt + 1) * P],
                                        rhs=qkts[po:po + HD, hp, ch * CHW + c0:(ch + 1) * CHW],
                                        start=True,
                                        stop=True,
                                    )
                                    sl += w[kt]
                                nc.scalar.activation(
                                    out=pt[:, par, off[s0]:off[s0] + sw],
                                    in_=sp[:, 0:sw],
                                    func=Exp,
                                    scale=0.125,
                                )
                                for kt in range(s0, s1):
                                    if kt >= 4 * ch:
                                        o = off[kt]
                                        nc.vector.tensor_mul(
                                            out=pt[:, par, o:o + P],
                                            in0=pt[:, par, o:o + P],
                                            in1=tri[:, :],
                                        )
                            pace()

                        # AV (q-major, fused denominator): all 8 chains first,
                        # then all 8 transposes, so the PE never waits on the
                        # DVE reciprocal+scale chain of the tile it just made.
                        pace(3)
                        yt = bpool.tile([P, CHW], BF16, tag="B", name=f"yt_{rtag}_{hp}_{ch}")
                        ybs = {}
                        for par in (0, 1):
                            if par:
                                pace(3)
                            hl = 2 * hp + par
                            av = apool.tile(
                                [P, 4, HD + 1], F32, tag="av",
                                name=f"av_{rtag}_{hp}_{ch}_{par}",
                            )
                            for qt in range(4):
                                last = 4 * ch + qt
                                for kt in range(last + 1):
                                    c0 = CHW - w[kt]
                                    sl = off[kt] + P * qt - c0
                                    nc.tensor.matmul(
                                        av[:, qt, :],
                                        lhsT=pt[:, par, sl:sl + P],
                                        rhs=vs[:, kt, hl, :],
                                        start=(kt == 0),
                                        stop=(kt == last),
                                    )
                                rec = wpool.tile(
                                    [P, 1], F32, tag="rec", bufs=4,
                                    name=f"rec_{rtag}_{hp}_{ch}_{par}_{qt}",
                                )
                                nc.vector.reciprocal(rec[:, :], av[:, qt, HD:HD + 1])
                                yb = wpool.tile(
                                    [P, HD], BF16, tag="yb", bufs=10,
                                    name=f"yb_{rtag}_{hp}_{ch}_{par}_{qt}",
                                )
                                nc.vector.tensor_scalar_mul(
                                    out=yb[:, :], in0=av[:, qt, 0:HD], scalar1=rec[:, 0:1]
                                )
                                ybs[(par, qt)] = yb
                        for par in (0, 1):
                            for qt in range(4):
                                nc.tensor.transpose(
                                    out=yt[64 * par:64 * par + HD, qt * P:(qt + 1) * P],
                                    in_=ybs[(par, qt)][:, :],
                                    identity=iden[:, :],
                                )
                        nc.vector.tensor_copy(
                            out=yts[:, hp, ch * CHW:(ch + 1) * CHW], in_=yt
                        )
                        pace()

                    for tt in range(4 * ch, 4 * ch + 4):
                        for cc in range(2):
                            outq.append((emit_out, (tt, cc)))

                while projq:
                    f, a = projq.popleft()
                    f(*a)
                # final out-proj groups: evict on the now-idle Act engine
                while outq:
                    f, (tt, cc) = outq.popleft()
                    f(tt, cc, True)

    nc.finalize()
    return nc


def _core_inputs(x, w_qkv, b_qkv, w_out, core, tri, iden):
    b, g = core // 2, core % 2
    qc = slice(G * g, G * g + G)
    kc = slice(C + G * g, C + G * g + G)
    vc = slice(2 * C + G * g, 2 * C + G * g + G)
    xt = np.ascontiguousarray(x[b].T).astype(bfloat16)
    wqk = np.ascontiguousarray(
        np.concatenate([w_qkv[:, qc], w_qkv[:, kc]], axis=1)
    ).astype(bfloat16)
    bqk = np.ascontiguousarray(
        np.concatenate([b_qkv[qc], b_qkv[kc]]).reshape(8, P).T
    ).astype(np.float32)
    wv = np.ascontiguousarray(w_qkv[:, vc]).astype(bfloat16)
    bvb = np.ascontiguousarray(
        np.broadcast_to(b_qkv[vc].reshape(1, 8, HD), (P, 8, HD))
    ).astype(bfloat16)
    wo = np.ascontiguousarray(
        w_out[G * g:G * g + G, :].reshape(4, P, C).transpose(1, 0, 2)
    ).astype(bfloat16)
    return {
        "xt": xt,
        "wqk": wqk,
        "bqk": bqk,
        "wv": wv,
        "bvb": bvb,
        "wo": wo,
        "tri": tri,
        "iden": iden,
    }


def kernel(x, w_qkv, b_qkv, w_out, b_out):
    global _BUILT, LAST_EXEC_TIME_NS
    x = np.asarray(x, dtype=np.float32)
    w_qkv = np.asarray(w_qkv, dtype=np.float32)
    b_qkv = np.asarray(b_qkv, dtype=np.float32)
    w_out = np.asarray(w_out, dtype=np.float32)
    b_out = np.asarray(b_out, dtype=np.float32)

    if _BUILT is None:
        _BUILT = _build_bass()
    nc = _BUILT

    p = np.arange(P)
    tri = (p[:, None] <= p[None, :]).astype(bfloat16)
    iden = np.eye(P, dtype=bfloat16)
    in_maps = [
        _core_inputs(x, w_qkv, b_qkv, w_out, core, tri, iden) for core in range(8)
    ]
    trace = bool(int(os.environ.get("KERNEL_TRACE", "0")))
    res = run_bass_kernel_spmd(nc, in_maps, list(range(8)), trace=trace)
    LAST_EXEC_TIME_NS = res.exec_time_ns

    out = np.empty((B, T, C), dtype=np.float32)
    for b in range(B):
        out[b] = res.results[2 * b]["out"] + res.results[2 * b + 1]["out"] + b_out
    return out


# revision 30
# speedup vs baseline: 1.5970x; 1.0542x over previous
"""Causal self-attention (B=4, T=2048, C=1024, H=16) on 8 trn2 NeuronCores.

Sharding: core = (batch b, head-group g) with b = core//2, g = core%2.
Each core handles one batch and 8 heads (column-parallel qkv, row-parallel
out_proj).  Cores return partial out-projection results; the host sums the
two head-group partials per batch and adds b_out.

Per-core kernel (all matmuls bf16 with fp32 PSUM accumulate), restructured
for PE occupancy and minimal PE work:
  - streaming startup: x^T loaded in four T-chunks, w_qkv in eight 128-col
    blocks (HWDGE/SP queue), so the first projection matmuls start ~4us in.
  - per q-chunk ch (512 cols): qk projection for that chunk, V projection
    for its four 128-row tiles, then attention for the four head pairs, then
    the chunk's out-projection — out-proj and next-chunk projections are
    emitted as PE "filler" quanta interleaved between score spans so the PE
    stays busy while the Activation engine drains exp() work.
  - scores computed transposed S^T[k, q] with ragged causal widths (the
    k-tiles near the diagonal only compute the surviving q columns).
  - p = exp(0.125 * S^T) on the scalar engine on packed 2-k-tile spans;
    diagonal 128x128 triangles masked by a single precomputed 0/1 mask.
  - AV computed q-major: out[q, d] = sum_k P^T[k,q]^T V[k,d] with a fused
    ones column in V giving the softmax denominator in column 64 — output
    free size is 65 instead of 512, a ~2.3x cut in AV PE time.
  - normalization per-partition (reciprocal + tensor_scalar_mul on DVE),
    then a PE transpose (identity matmul) back to d-major y^T for the
    row-parallel out projection.
"""

import os
from collections import deque

import numpy as np

try:
    import concourse.bass as bass
except ImportError:  # pragma: no cover
    import sys

    sys.path.insert(0, "/opt/trn_rl_repo")
    import concourse.bass as bass

import concourse.mybir as mybir
from concourse.bacc import Bacc
from concourse.bass_utils import run_bass_kernel_spmd
from concourse.tile import TileContext
from ml_dtypes import bfloat16

B, T, C, H = 4, 2048, 1024, 16
HD = 64        # head dim
G = 512        # head-group width: 8 heads * 64
P = 128
NCT = C // P   # contraction tiles over C
NTT = T // P   # 128-row tiles over T
CHW = 512      # q-chunk width
NCH = T // CHW

F32 = mybir.dt.float32
BF16 = mybir.dt.bfloat16
FP8 = mybir.dt.float8e4

_BUILT = None
LAST_EXEC_TIME_NS = None


def _widths(ch):
    """Per-k-tile ragged score widths and pt-column offsets for chunk ch."""
    nkt = 4 * ch + 4
    w = [CHW - max(0, P * (kt - 4 * ch)) for kt in range(nkt)]
    off = [0]
    for x in w:
        off.append(off[-1] + x)
    return nkt, w, off


PTW = _widths(NCH - 1)[2][-1]  # widest pt row (chunk 3): 7424


def _build_bass(reps=1):
    nc = Bacc("TRN2", target_bir_lowering=False)

    xt_d = nc.dram_tensor("xt", [C, T], BF16, kind="ExternalInput")
    wqk_d = nc.dram_tensor("wqk", [C, 2 * G], BF16, kind="ExternalInput")
    bqk_d = nc.dram_tensor("bqk", [P, 8], F32, kind="ExternalInput")
    wv_d = nc.dram_tensor("wv", [C, G], BF16, kind="ExternalInput")
    bvb_d = nc.dram_tensor("bvb", [P, 8, HD], BF16, kind="ExternalInput")
    wo_d = nc.dram_tensor("wo", [P, 4, C], BF16, kind="ExternalInput")
    tri_d = nc.dram_tensor("tri", [P, P], BF16, kind="ExternalInput")
    iden_d = nc.dram_tensor("iden", [P, P], BF16, kind="ExternalInput")
    out_d = nc.dram_tensor("out", [T, C], BF16, kind="ExternalOutput")

    Exp = mybir.ActivationFunctionType.Exp

    with TileContext(nc) as tc:
        with (
            tc.tile_pool(name="const", bufs=1) as cpool,
            tc.tile_pool(name="work", bufs=3) as wpool,
            tc.tile_pool(name="bpool", bufs=2, space="PSUM") as bpool,
            tc.tile_pool(name="spool", bufs=2, space="PSUM") as spool,
            tc.tile_pool(name="apool", bufs=2, space="PSUM") as apool,
        ):
            xts = cpool.tile([P, NCT, T], BF16, tag="xts")
            wqks = cpool.tile([P, NCT, 2 * G], BF16, tag="wqks")
            bqks = cpool.tile([P, 8], F32, tag="bqks")
            wvs = cpool.tile([P, NCT, G], BF16, tag="wvs")
            bvb = cpool.tile([P, 8, HD], BF16, tag="bvb")
            wos = cpool.tile([P, 4, C], BF16, tag="wos")
            tri = cpool.tile([P, P], BF16, tag="tri")
            iden = cpool.tile([P, P], BF16, tag="iden")
            q8 = cpool.tile([P, 2, 2, T], FP8, tag="q8")
            k8 = cpool.tile([P, 2, 2, T], FP8, tag="k8")
            vs = cpool.tile([P, NTT, 8, HD + 1], BF16, tag="vs")
            yts = cpool.tile([P, 4, T], BF16, tag="yts")
            pts = [
                cpool.tile([P, 2, PTW], BF16, tag=f"pt{i}", name=f"pt{i}")
                for i in range(2)
            ]

            xr = xt_d.rearrange("(a p) t -> p a t", p=P)
            wr = wqk_d.rearrange("(a p) m -> p a m", p=P)
            # DMA issue order ~ readiness order of first consumers.  The first
            # x chunk is split per contraction tile so the first qk projection
            # matmuls can start as soon as tile 0 lands.
            nc.sync.dma_start(out=bqks[:, :], in_=bqk_d[:, :])
            nc.sync.dma_start(out=wqks[:, :, 0:P], in_=wr[:, :, 0:P])
            for ct in range(2):
                nc.sync.dma_start(
                    out=xts[:, ct, 0:CHW], in_=xr[:, ct, 0:CHW]
                )
            nc.sync.dma_start(out=wqks[:, :, 4 * P:5 * P], in_=wr[:, :, 4 * P:5 * P])
            for ct in range(2, NCT):
                nc.sync.dma_start(
                    out=xts[:, ct, 0:CHW], in_=xr[:, ct, 0:CHW]
                )
            nc.sync.dma_start(out=tri[:, :], in_=tri_d[:, :])
            nc.sync.dma_start(out=iden[:, :], in_=iden_d[:, :])
            for mt in (1, 5, 2, 6, 3, 7):
                nc.sync.dma_start(
                    out=wqks[:, :, mt * P:(mt + 1) * P], in_=wr[:, :, mt * P:(mt + 1) * P]
                )
            nc.sync.dma_start(out=wvs[:, :, :], in_=wv_d.rearrange("(a p) m -> p a m", p=P))
            nc.sync.dma_start(out=bvb[:, :, :], in_=bvb_d[:, :, :])
            for c in (1, 2, 3):
                nc.sync.dma_start(
                    out=xts[:, :, c * CHW:(c + 1) * CHW], in_=xr[:, :, c * CHW:(c + 1) * CHW]
                )
            nc.sync.dma_start(out=wos[:, :, :], in_=wo_d[:, :, :])

            # PE p-state warmup: a stream of tiny matmuls on memset data,
            # issued before any DMA-dependent work, keeps the tensor engine
            # clock ramping from t=0 so the first projections run full speed.
            warm = wpool.tile([1, P], BF16, tag="warm", bufs=1, name="warm")
            nc.vector.memset(warm[:, :], 1.0)
            wps = apool.tile([P, 4, HD + 1], F32, tag="av", name="warm_ps")
            for i in range(46):
                nc.tensor.matmul(
                    wps[0:HD, 0, :],
                    lhsT=warm[0:1, 0:HD],
                    rhs=warm[0:1, 0:HD + 1],
                    start=True,
                    stop=True,
                )

            for rep in range(reps):
                rtag = f"r{rep}"

                def _qk_dst(blk, parity, ch):
                    dst = q8 if blk < 2 else k8
                    return dst[:, blk % 2, parity, ch * CHW:(ch + 1) * CHW]

                def _qk_evict(ps, blk, parity, ch, on_act):
                    if on_act:
                        nc.scalar.activation(
                            out=_qk_dst(blk, parity, ch),
                            in_=ps,
                            func=mybir.ActivationFunctionType.Identity,
                            bias=bqks[:, 2 * blk + parity:2 * blk + parity + 1],
                        )
                    else:
                        nc.vector.tensor_scalar_add(
                            out=_qk_dst(blk, parity, ch),
                            in0=ps,
                            scalar1=bqks[:, 2 * blk + parity:2 * blk + parity + 1],
                        )

                def emit_qk(blk, parity, ch):
                    c0 = 256 * blk + 128 * parity
                    ps = bpool.tile([P, CHW], F32, tag="B", name=f"qk_{rtag}_{blk}_{parity}_{ch}")
                    for ct in range(NCT):
                        nc.tensor.matmul(
                            ps,
                            lhsT=wqks[:, ct, c0:c0 + P],
                            rhs=xts[:, ct, ch * CHW:(ch + 1) * CHW],
                            start=(ct == 0),
                            stop=(ct == NCT - 1),
                        )
                    _qk_evict(ps, blk, parity, ch, ch == 0)

                def emit_qk2(blk, ch):
                    # both parities of a qk block with interleaved contraction
                    # matmuls, so the second doesn't wait out the x-tile DMAs
                    pss = []
                    for parity in (0, 1):
                        pss.append(bpool.tile(
                            [P, CHW], F32, tag="B", name=f"qk_{rtag}_{blk}_{parity}_{ch}"
                        ))
                    for ct in range(NCT):
                        for parity in (0, 1):
                            c0 = 256 * blk + 128 * parity
                            nc.tensor.matmul(
                                pss[parity],
                                lhsT=wqks[:, ct, c0:c0 + P],
                                rhs=xts[:, ct, ch * CHW:(ch + 1) * CHW],
                                start=(ct == 0),
                                stop=(ct == NCT - 1),
                            )
                    for parity in (0, 1):
                        _qk_evict(pss[parity], blk, parity, ch, True)

                def emit_v(tt):
                    ps = bpool.tile([P, CHW], F32, tag="B", name=f"v_{rtag}_{tt}")
                    for ct in range(NCT):
                        nc.tensor.matmul(
                            ps,
                            lhsT=xts[:, ct, tt * P:(tt + 1) * P],
                            rhs=wvs[:, ct, :],
                            start=(ct == 0),
                            stop=(ct == NCT - 1),
                        )
                    nc.vector.memset(vs[:, tt, :, HD:HD + 1], 1.0)
                    nc.vector.tensor_add(
                        out=vs[:, tt, :, 0:HD],
                        in0=ps.rearrange("p (h d) -> p h d", h=8),
                        in1=bvb[:, :, :],
                    )

                def emit_out(tt, cc, on_act=False):
                    ps = bpool.tile([P, CHW], F32, tag="B", name=f"o_{rtag}_{tt}_{cc}")
                    for j in range(4):
                        nc.tensor.matmul(
                            ps,
                            lhsT=yts[:, j, tt * P:(tt + 1) * P],
                            rhs=wos[:, j, cc * CHW:(cc + 1) * CHW],
                            start=(j == 0),
                            stop=(j == 3),
                        )
                    ev = wpool.tile([P, CHW], BF16, tag="ev", name=f"ev_{rtag}_{tt}_{cc}")
                    if on_act:
                        nc.scalar.activation(
                            out=ev[:, :], in_=ps,
                            func=mybir.ActivationFunctionType.Copy,
                        )
                    else:
                        nc.vector.tensor_copy(out=ev[:, :], in_=ps)
                    nc.sync.dma_start(
                        out=out_d[tt * P:(tt + 1) * P, cc * CHW:(cc + 1) * CHW],
                        in_=ev[:, :],
                    )

                projq = deque()
                outq = deque()
                projq.append((emit_qk2, (0, 0)))
                projq.append((emit_qk2, (2, 0)))
                for blk in (1, 3):
                    for parity in (0, 1):
                        projq.append((emit_qk, (blk, parity, 0)))
                for tt in range(4):
                    projq.append((emit_v, (tt,)))

                for ch in range(NCH):
                    while projq:
                        f, a = projq.popleft()
                        f(*a)
                    if ch + 1 < NCH:
                        for blk in (0, 2, 1, 3):
                            for parity in (0, 1):
                                projq.append((emit_qk, (blk, parity, ch + 1)))
                        for tt in range(4 * (ch + 1), 4 * (ch + 1) + 4):
                            projq.append((emit_v, (tt,)))

                    # out-proj quanta are the only filler available during the
                    # last chunk (no next-chunk projections), which is also
                    # where the Activation engine's exp backlog is largest —
                    # hold back a reserve of them on earlier chunks.
                    keep = 8 * ch if ch + 1 < NCH else 0

                    def drain_one():
                        if projq:
                            f, a = projq.popleft()
                            f(*a)
                            return True
                        if len(outq) > keep:
                            f, a = outq.popleft()
                            f(*a)
                            return True
                        return False

                    nkt, w, off = _widths(ch)
                    nspans = -(-nkt // 2)
                    # pace filler drains across this chunk's drain slots;
                    # the slots around the AV chains (where the PE waits on
                    # the exp backlog) get extra weight
                    nslots = 4 * (nspans + 7)
                    nquanta = len(projq) + max(0, len(outq) - keep)
                    drained = [0]
                    slot = [0]

                    def pace(wt=1):
                        slot[0] += wt
                        want = (nquanta * slot[0] + nslots - 1) // nslots
                        while drained[0] < want and drain_one():
                            drained[0] += 1

                    for hp in range(4):
                        pt = pts[hp % 2]
                        # scores + exp + diagonal masks, spans of 2 k-tiles
                        for s0 in range(0, nkt, 2):
                            s1 = min(s0 + 2, nkt)
                            for par in (0, 1):
                                po = 64 * par
                                sw = off[s1] - off[s0]
                                sp = spool.tile(
                                    [P, 2 * CHW], F32, tag="s",
                                    name=f"sp_{rtag}_{hp}_{ch}_{s0}_{par}",
                                )
                                h = 2 * hp + par
                                g, m = h // 4, 32 * (h % 4)
                                sl = 0
                                for kt in range(s0, s1):
                                    c0 = CHW - w[kt]
                                    nc.tensor.matmul(
                                        sp[:, sl:sl + w[kt]],
                                        lhsT=k8[m:m + 32, g, :, kt * P:(kt + 1) * P],
                                        rhs=q8[m:m + 32, g, :, ch * CHW + c0:(ch + 1) * CHW],
                                        start=True,
                                        stop=True,
                                        perf_mode=mybir.MatmulPerfMode.DoubleRow,
                                        tile_position=(m, 0),
                                    )
                                    sl += w[kt]
                                nc.scalar.activation(
                                    out=pt[:, par, off[s0]:off[s0] + sw],
                                    in_=sp[:, 0:sw],
                                    func=Exp,
                                    scale=0.125,
                                )
                                for kt in range(s0, s1):
                                    if kt >= 4 * ch:
                                        o = off[kt]
                                        nc.vector.tensor_mul(
                                            out=pt[:, par, o:o + P],
                                            in0=pt[:, par, o:o + P],
                                            in1=tri[:, :],
                                        )
                            pace()

                        # AV (q-major, fused denominator): all 8 chains first,
                        # then all 8 transposes, so the PE never waits on the
                        # DVE reciprocal+scale chain of the tile it just made.
                        pace(3)
                        yt = bpool.tile([P, CHW], BF16, tag="B", name=f"yt_{rtag}_{hp}_{ch}")
                        ybs = {}
                        for par in (0, 1):
                            if par:
                                pace(3)
                            hl = 2 * hp + par
                            av = apool.tile(
                                [P, 4, HD + 1], F32, tag="av",
                                name=f"av_{rtag}_{hp}_{ch}_{par}",
                            )
                            for qt in range(4):
                                last = 4 * ch + qt
                                for kt in range(last + 1):
                                    c0 = CHW - w[kt]
                                    sl = off[kt] + P * qt - c0
                                    nc.tensor.matmul(
                                        av[:, qt, :],
                                        lhsT=pt[:, par, sl:sl + P],
                                        rhs=vs[:, kt, hl, :],
                                        start=(kt == 0),
                                        stop=(kt == last),
                                    )
                                rec = wpool.tile(
                                    [P, 1], F32, tag="rec", bufs=4,
                                    name=f"rec_{rtag}_{hp}_{ch}_{par}_{qt}",
                                )
                                nc.vector.reciprocal(rec[:, :], av[:, qt, HD:HD + 1])
                                yb = wpool.tile(
                                    [P, HD], BF16, tag="yb", bufs=10,
                                    name=f"yb_{rtag}_{hp}_{ch}_{par}_{qt}",
                                )
                                nc.vector.tensor_scalar_mul(
                                    out=yb[:, :], in0=av[:, qt, 0:HD], scalar1=rec[:, 0:1]
                                )
                                ybs[(par, qt)] = yb
                        for par in (0, 1):
                            for qt in range(4):
                                nc.tensor.transpose(
                                    out=yt[64 * par:64 * par + HD, qt * P:(qt + 1) * P],
                                    in_=ybs[(par, qt)][:, :],
                                    identity=iden[:, :],
                                )
                        nc.vector.tensor_copy(
                            out=yts[:, hp, ch * CHW:(ch + 1) * CHW], in_=yt
                        )
                        pace()

                    for tt in range(4 * ch, 4 * ch + 4):
                        for cc in range(2):
                            outq.append((emit_out, (tt, cc)))

                while projq:
                    f, a = projq.popleft()
                    f(*a)
                # final out-proj groups: evict on the now-idle Act engine
                while outq:
                    f, (tt, cc) = outq.popleft()
                    f(tt, cc, True)

    nc.finalize()
    return nc


def _qk_perm():
    """Column permutation: within each 256-col (4-head) block, even dims of
    the 4 heads first (32 per head), then odd dims — the DoubleRow layout."""
    idx = np.empty(2 * G, np.int64)
    for blk in range(4):
        base = 256 * blk
        for parity in range(2):
            for h4 in range(4):
                idx[base + 128 * parity + 32 * h4:base + 128 * parity + 32 * h4 + 32] = (
                    base + 64 * h4 + 2 * np.arange(32) + parity
                )
    return idx


def _core_inputs(x, w_qkv, b_qkv, w_out, core, tri, iden):
    b, g = core // 2, core % 2
    qc = slice(G * g, G * g + G)
    kc = slice(C + G * g, C + G * g + G)
    vc = slice(2 * C + G * g, 2 * C + G * g + G)
    xt = np.ascontiguousarray(x[b].T).astype(bfloat16)
    perm = _qk_perm()
    wqk = np.ascontiguousarray(
        np.concatenate([w_qkv[:, qc], w_qkv[:, kc]], axis=1)[:, perm]
    ).astype(bfloat16)
    bqk_full = np.concatenate([b_qkv[qc], b_qkv[kc]])
    p = np.arange(P)
    bqk = np.empty((P, 8), np.float32)
    for blk in range(4):
        for parity in range(2):
            bqk[:, 2 * blk + parity] = bqk_full[
                256 * blk + 64 * (p // 32) + 2 * (p % 32) + parity
            ]
    bqk = np.ascontiguousarray(bqk)
    wv = np.ascontiguousarray(w_qkv[:, vc]).astype(bfloat16)
    bvb = np.ascontiguousarray(
        np.broadcast_to(b_qkv[vc].reshape(1, 8, HD), (P, 8, HD))
    ).astype(bfloat16)
    wo = np.ascontiguousarray(
        w_out[G * g:G * g + G, :].reshape(4, P, C).transpose(1, 0, 2)
    ).astype(bfloat16)
    return {
        "xt": xt,
        "wqk": wqk,
        "bqk": bqk,
        "wv": wv,
        "bvb": bvb,
        "wo": wo,
        "tri": tri,
        "iden": iden,
    }


def kernel(x, w_qkv, b_qkv, w_out, b_out):
    global _BUILT, LAST_EXEC_TIME_NS
    x = np.asarray(x, dtype=np.float32)
    w_qkv = np.asarray(w_qkv, dtype=np.float32)
    b_qkv = np.asarray(b_qkv, dtype=np.float32)
    w_out = np.asarray(w_out, dtype=np.float32)
    b_out = np.asarray(b_out, dtype=np.float32)

    if _BUILT is None:
        _BUILT = _build_bass()
    nc = _BUILT

    p = np.arange(P)
    tri = (p[:, None] <= p[None, :]).astype(bfloat16)
    iden = np.eye(P, dtype=bfloat16)
    in_maps = [
        _core_inputs(x, w_qkv, b_qkv, w_out, core, tri, iden) for core in range(8)
    ]
    trace = bool(int(os.environ.get("KERNEL_TRACE", "0")))
    res = run_bass_kernel_spmd(nc, in_maps, list(range(8)), trace=trace)
    LAST_EXEC_TIME_NS = res.exec_time_ns

    out = np.empty((B, T, C), dtype=np.float32)
    for b in range(B):
        out[b] = res.results[2 * b]["out"] + res.results[2 * b + 1]["out"] + b_out
    return out
